# revision 61
# baseline (speedup 1.0000x reference)
"""MoE (DeepSeek-style, no gate) SwiGLU kernel for 8 Trainium2 NeuronCores.

Strategy (expert parallelism, per the sharding hint):
  - 16 routed experts sharded 2-per-core across 8 cores.
  - Token dispatch done host-side: for each expert, gather the tokens routed
    to it (topk membership), pad to a common capacity, and ship the
    pre-transposed activation columns xT[:, tokens] to the owning core.
  - Shared expert tensor-parallel over its inter dim (2816/2 halves) x
    token quarters, computed on all 2048 tokens in bf16.
  - Each core runs the same Bass program (SPMD) on its own shard; the host
    scatter-adds routed outputs and sums shared-expert partials.

Two precision tiers for routed tokens (tier by combine weight):
  - comb >= T_FP8: bf16 pipeline (PE at 1 col/cycle).  The output error
    contribution of a routed pair scales with comb, so large-comb pairs
    need bf16.
  - comb <  T_FP8: fp8 e4m3 pipeline with perf_mode=DoubleRow (2 k-tiles
    per matmul, ~1.4x PE throughput).  All quantization scales are powers
    of two folded into ACT scale operands and the host-side combine
    weights, so the fp8 path has the same instruction structure as bf16.
  Measured (numpy sim of e4m3-quantized pipeline on the real inputs):
  T_FP8=0.45 -> rel err ~1.4e-2 vs the 2e-2 gate; all-bf16 is 3.9e-3.

Compute layout per phase (one phase = one SwiGLU MLP on a token set):
  phase 1:  h1T = W1p.T @ xT, h3T = W3p.T @ xT   (I on partitions, tokens free)
            h' = silu(h1T) * h3T                  -> bf16 (or e4m3) in SBUF
  phase 2:  y[tok_tile] = h'.T @ W2p, scaled per-token by comb weight on
            PSUM eviction (DVE mul with a broadcast comb row).

Overlap structure (each item measured against the 437us all-bf16 baseline;
this version: ~430us, rel err 1.46e-2):
  - Phase interleave e0,f0,e1,f1,shared: big bf16 weight prefetches overlap
    an fp8 phase's compute in addition to a phase 2, and buffer-tag
    rotations (w13/wf13) get a whole phase of slack before reuse.
  - Per-phase alternating A/B tags (bufs=1) for xt/xf/cb: every phase's
    input DMAs are first-use, so the serial SP HWDGE ring streams them in
    demand order instead of head-of-line blocking on buffer-recycle waits.
  - Software-pipelined emission (generators): each phase's DMA header is
    emitted right after the previous phase's phase-1 panels, ahead of its
    w2 bulk on the ring.
  - Output DMAs ride the ACT ring: on the SP ring they pace the next
    phase's inputs to phase-2 compute.
  - Ramp: 11 dummy warm-up matmuls at t=0 keep the PE HAM activity monitor
    busy during the initial DMA wait; the first phase's m=0 W13 panel is
    split into 4 k-group DMAs interleaved with the xt token groups in
    SP-ring order, first two xt groups on the then-idle ACT ring.
Measured dead ends: fp8 for the shared expert or large-comb pairs fails the
2e-2 gate; dropping small-comb pairs outright fails (max-abs metric); int8
matmul unsupported by bass; >2 ACT-ring ramp DMAs stall the PE; w2/wf2 on
the gpsimd SWDGE ring (pushes at t=0 steal ramp HBM bw) or the ACT ring
(gated behind ph1 ACTIVATEs, arrive too late); psA=3/psY=2 PSUM split;
hoisting headers 2 phases early (NaN race); routing later phases' m=0/m=1
through the per-kg ramp tags (scheduler gates w2 pushes on late matmul
counters).
"""

import numpy as np
import ml_dtypes

import concourse.bass as bass
import concourse.bacc as bacc
import concourse.mybir as mybir
import concourse.tile as tile
from concourse.bass_utils import run_bass_kernel_spmd

BF16 = ml_dtypes.bfloat16
F8 = ml_dtypes.float8_e4m3
F32 = np.float32
P = 128
NSZ = 512   # PSUM bank free size (fp32)
XG = 2      # xt k-tiles per DMA group (= one DoubleRow pair)
WKG = 4     # w13 k-tiles per k-group block (bf16 path)

# fp8 tier: threshold and power-of-two quantization scales.
T_FP8 = 0.45
SX = 32.0       # x -> e4m3 scale
SW = 1024.0     # W1/W3 -> e4m3 scale
SH = 8.0        # h' -> e4m3 scale
SW2 = 1024.0    # W2 -> e4m3 scale
S1 = SX * SW                  # PSUM p1/p3 carry h*S1
K_COPY = SH / (S1 * S1)       # ACT copy scale: c3 = p3 * K_COPY
KP8 = 8                       # phase-1 k-tile pairs (D/128/2)
JP8 = 6                       # phase-2 I-tile pairs (11 -> pad to 12)

FULL_CFG = dict(
    ncores=8,
    T=2048,
    D=2048,
    E=16,
    I_E=1408,
    sh_half=1408,    # shared expert sharded 2 (inter) x ncores/2 (tokens)
    d_out=2048,
)


def _derived(cfg):
    nt = max(1, cfg["ncores"] // 2)
    return dict(
        epc=cfg["E"] // cfg["ncores"],
        kd=cfg["D"] // P,
        it_r=cfg["I_E"] // P,
        it_s=cfg["sh_half"] // P,
        n_tok_shards=nt,
        sh_tok=cfg["T"] // nt,
    )


def _emit_phase(nc, pools, xt_dram, w13_dram, w2_dram, cb_dram,
                out_rows, n_itiles, cp, cfg, ph, grp, first=False):
    """One bf16 SwiGLU MLP phase over `cp` token columns, `n_itiles` I-tiles.

    Output is TRANSPOSED: out_rows[mt2] is the DRAM destination for model-dim
    tile mt2 ([128, cp] = [D-tile, tokens]).  cb_dram is None for the shared
    expert; otherwise the combine weight broadcast to [128, cp].  `first`
    marks the program's first phase: its m=0 W13 panel DMA is split per
    k-group and interleaved with the xt groups for the ramp.
    """
    kd = _derived(cfg)["kd"]
    d_out = cfg["d_out"]
    dt = mybir.dt.bfloat16
    f32 = mybir.dt.float32
    nkg = kd // WKG

    xtp, wp, hpp, w2p, sp, op, cgp, psA, psY = (
        pools["xt"], pools["w"], pools["hp"], pools["w2"], pools["s"],
        pools["o"], pools["cg"], pools["psA"], pools["psY"])

    # DMA header. The SP HWDGE ring executes DMAs serially in issue order,
    # so for the program's first phase the m=0 panel k-groups and the xt
    # token-k groups are interleaved to match the matmul consumption order.
    xtg = []
    if first:
        wkg = [wp.tile([P, 2, WKG, P], dt, tag=f"w13a{kg}", bufs=2,
                       name=f"w13a_{ph}_{kg}") for kg in range(nkg)]
        wpre = [wkg]
        for g in range(kd // XG):
            xtg.append(xtp.tile([P, XG, cp], dt, tag=f"xt{grp}_{g}",
                                name=f"xt_{ph}_{g}"))
        # First two xt groups ride the otherwise-idle ACT HWDGE ring.
        nc.scalar.dma_start(out=xtg[0][:], in_=xt_dram[0])
        nc.scalar.dma_start(out=xtg[1][:], in_=xt_dram[1])
        # SP ring in PE demand order, weights one k-group ahead.
        for kind, i in (("w", 0), ("w", 1), ("x", 2), ("x", 3), ("w", 2),
                        ("x", 4), ("x", 5), ("w", 3), ("x", 6), ("x", 7)):
            if kind == "w":
                nc.sync.dma_start(out=wkg[i][:], in_=w13_dram[0][:, i])
            else:
                nc.sync.dma_start(out=xtg[i][:], in_=xt_dram[i])
    else:
        wpre = [wp.tile([P, nkg, 2, WKG, P], dt, tag="w13", name=f"w13_{ph}_0")]
        nc.sync.dma_start(out=wpre[0][:], in_=w13_dram[0])
        for g in range(kd // XG):
            xge = xtp.tile([P, XG, cp], dt, tag=f"xt{grp}_{g}",
                           name=f"xt_{ph}_{g}")
            nc.sync.dma_start(out=xge[:], in_=xt_dram[g])
            xtg.append(xge)
    if n_itiles > 1:
        # (Measured dead end: routing phases' m=0/m=1 through per-kg ramp
        # tags made the scheduler gate this phase's w2 pushes on late-phase
        # matmul counters -> 12us stall mid phase-2.)
        w13b = wp.tile([P, nkg, 2, WKG, P], dt, tag="w13", name=f"w13_{ph}_1")
        nc.sync.dma_start(out=w13b[:], in_=w13_dram[1])
        wpre.append(w13b)

    cbt = None
    if cb_dram is not None:
        cbr = cgp.tile([P, cp], f32, tag=f"cbr{grp}", name=f"cbr_{ph}")
        nc.sync.dma_start(out=cbr[:], in_=cb_dram[:])
        # Bounce through DVE so the per-tile eviction muls below need only
        # the PE wait (DVE has already observed the cb DMA here).
        cbt = cgp.tile([P, cp], f32, tag=f"cb{grp}", name=f"cb_{ph}")
        nc.vector.tensor_copy(cbt[:], cbr[:])

    yield  # header done (emitted one phase early so the ring prefetches it)

    # ---- phase 1: h' = silu(xW1) * (xW3), transposed layout [I, tokens] ----
    hp = []
    for m in range(n_itiles):
        if m < len(wpre):
            w13t = wpre[m]
        else:
            w13t = wp.tile([P, nkg, 2, WKG, P], dt, tag="w13", name=f"w13_{ph}_{m}")
            nc.sync.dma_start(out=w13t[:], in_=w13_dram[m])
        hpm = hpp.tile([P, cp], dt, tag=f"hp_{m}", name=f"hp_{ph}_{m}")
        for n0 in range(0, cp, NSZ):
            nsz = min(NSZ, cp - n0)
            p1 = psA.tile([P, nsz], f32, tag="p1", name=f"p1_{ph}_{m}_{n0}")
            p3 = psA.tile([P, nsz], f32, tag="p3", name=f"p3_{ph}_{m}_{n0}")
            for kt in range(kd):
                xs = xtg[kt // XG][:, kt % XG, n0:n0 + nsz]
                if isinstance(w13t, list):   # first phase m=0/1: per-kg tiles
                    w1s = w13t[kt // WKG][:, 0, kt % WKG, :]
                    w3s = w13t[kt // WKG][:, 1, kt % WKG, :]
                else:
                    w1s = w13t[:, kt // WKG, 0, kt % WKG, :]
                    w3s = w13t[:, kt // WKG, 1, kt % WKG, :]
                nc.tensor.matmul(p1[:], w1s, xs,
                                 start=(kt == 0), stop=(kt == kd - 1))
                nc.tensor.matmul(p3[:], w3s, xs,
                                 start=(kt == 0), stop=(kt == kd - 1))
            # silu(h1)*h3 = sigmoid(h1)*h3*h1.
            s = sp.tile([P, nsz], f32, tag="s", name=f"s_{ph}_{m}_{n0}")
            nc.scalar.activation(s[:], p1[:],
                                 mybir.ActivationFunctionType.Sigmoid)
            c3 = sp.tile([P, nsz], f32, tag="c3", name=f"c3_{ph}_{m}_{n0}")
            nc.scalar.copy(c3[:], p3[:])
            t = sp.tile([P, nsz], f32, tag="t", name=f"t_{ph}_{m}_{n0}")
            nc.vector.tensor_mul(t[:], s[:], c3[:])
            nc.vector.tensor_mul(hpm[:, n0:n0 + nsz], t[:], p1[:])
        hp.append(hpm)

    yield  # phase 1 done

    # ---- phase 2: out[tok] = comb * (h'.T @ W2) ----
    # Phase-2 weights on the SP ring.  (Measured dead ends: gpsimd/SWDGE
    # pushes jump the queue at t=0 and steal HBM bandwidth from the ramp;
    # ACT-ring pushes are gated behind phase-1 ACTIVATEs and arrive too
    # late to prefetch.)
    w2t = []
    for kt in range(n_itiles):
        w = w2p.tile([P, d_out], dt, tag=f"w2_{kt}", name=f"w2_{ph}_{kt}")
        nc.sync.dma_start(out=w[:], in_=w2_dram[kt])
        w2t.append(w)

    ps2 = [(psY, "py"), (psY, "py"), (psY, "py"), (psY, "py"),
           (psA, "p1"), (psA, "p1"), (psA, "p3"), (psA, "p3")]
    idx = 0
    for mt2 in range(kd):
        osb = op.tile([P, cp], dt, tag="osb", name=f"osb_{ph}_{mt2}")
        for n0 in range(0, cp, NSZ):
            nn = min(NSZ, cp - n0)
            pool, ptag = ps2[idx % len(ps2)]
            idx += 1
            py = pool.tile([P, nn], f32, tag=ptag, name=f"py_{ph}_{mt2}_{n0}")
            for kt in range(n_itiles):
                nc.tensor.matmul(py[:], w2t[kt][:, mt2 * P:(mt2 + 1) * P],
                                 hp[kt][:, n0:n0 + nn],
                                 start=(kt == 0), stop=(kt == n_itiles - 1))
            if cbt is not None:
                nc.vector.tensor_mul(osb[:, n0:n0 + nn], py[:],
                                     cbt[:, n0:n0 + nn])
            elif idx % 2:
                nc.vector.tensor_copy(osb[:, n0:n0 + nn], py[:])
            else:
                nc.scalar.copy(osb[:, n0:n0 + nn], py[:])
        # Output DMAs ride the ACT ring: they are gated on eviction anyway,
        # and on the SP ring they pace the next phase's inputs to phase-2
        # compute (head-of-line blocking).
        nc.scalar.dma_start(out=out_rows[mt2], in_=osb[:])


def _emit_phase_f8(nc, pools, xf_dram, wf13_dram, wf2_dram, cf_dram,
                   out_rows, n_itiles, cp, cfg, ph, grp):
    """fp8 e4m3 DoubleRow SwiGLU phase over `cp` token columns.

    Same structure as the bf16 phase but every matmul is a DoubleRow pair
    (2 k-tiles per instruction).  PSUM p1/p3 carry h*S1; the quantization
    scales are folded into the ACT scale operands (phase 1) and into the
    host-prepared combine row cf = comb/(SH*SW2) (phase 2).  Phase 2 pads
    the 11 I-tiles to 12 with a zero 12th h' tile (and zero W2 rows).
    """
    kd = _derived(cfg)["kd"]
    d_out = cfg["d_out"]
    f8 = mybir.dt.float8e4
    dt = mybir.dt.bfloat16
    f32 = mybir.dt.float32

    xfp, wfp, hqp, w2fp, sp, op, cgp, psA, psY = (
        pools["xf"], pools["wf"], pools["hq"], pools["w2f"], pools["s"],
        pools["o"], pools["cg"], pools["psA"], pools["psY"])

    # DMA header: xt pair-groups + first two weight panels.
    xfg = []
    for g in range(kd // XG):
        xge = xfp.tile([P, XG, cp], f8, tag=f"xf{grp}_{g}", name=f"xf_{ph}_{g}")
        nc.sync.dma_start(out=xge[:], in_=xf_dram[g])
        xfg.append(xge)
    wpre = []
    for m in range(min(3, n_itiles)):
        wt = wfp.tile([P, KP8, 2, 2, P], f8, tag="wf13", bufs=3,
                      name=f"wf13_{ph}_{m}")
        nc.sync.dma_start(out=wt[:], in_=wf13_dram[m])
        wpre.append(wt)

    cfr = cgp.tile([P, cp], f32, tag=f"cfr{grp}", name=f"cfr_{ph}")
    nc.sync.dma_start(out=cfr[:], in_=cf_dram[:])
    cft = cgp.tile([P, cp], f32, tag=f"cf{grp}", name=f"cf_{ph}")
    nc.vector.tensor_copy(cft[:], cfr[:])

    yield  # header done

    # ---- phase 1: hq = e4m3(silu(h1) * h3 * SH), layout [I-tile, tokens] ----
    hq = hqp.tile([P, JP8 * 2, cp], f8, tag="hq", name=f"hq_{ph}")
    nc.gpsimd.memset(hq[:, n_itiles:, :], 0)   # zero-pad I-tiles 11..
    for m in range(n_itiles):
        if m < len(wpre):
            w13t = wpre[m]
        else:
            w13t = wfp.tile([P, KP8, 2, 2, P], f8, tag="wf13", bufs=3,
                            name=f"wf13_{ph}_{m}")
            nc.sync.dma_start(out=w13t[:], in_=wf13_dram[m])
        for n0 in range(0, cp, NSZ):
            nsz = min(NSZ, cp - n0)
            # Alternate m's accumulators onto the psY banks (idle during
            # phase 1): effective PSUM recycle depth 4, so the eviction
            # chain (~2us) stops gating the next m's matmuls (~2us/m here,
            # vs ~5.5us/m in the bf16 phases which don't need this).
            if m % 2 == 0:
                p1 = psA.tile([P, nsz], f32, tag="p1", name=f"q1_{ph}_{m}_{n0}")
                p3 = psA.tile([P, nsz], f32, tag="p3", name=f"q3_{ph}_{m}_{n0}")
            else:
                p1 = psY.tile([P, nsz], f32, tag="py", name=f"q1_{ph}_{m}_{n0}")
                p3 = psY.tile([P, nsz], f32, tag="py", name=f"q3_{ph}_{m}_{n0}")
            for kp in range(KP8):
                xs = xfg[kp][:, :, n0:n0 + nsz]
                nc.tensor.matmul(p1[:], w13t[:, kp, 0], xs,
                                 start=(kp == 0), stop=(kp == KP8 - 1),
                                 perf_mode=mybir.MatmulPerfMode.DoubleRow)
                nc.tensor.matmul(p3[:], w13t[:, kp, 1], xs,
                                 start=(kp == 0), stop=(kp == KP8 - 1),
                                 perf_mode=mybir.MatmulPerfMode.DoubleRow)
            # hq = sigmoid(p1/S1) * (p3*K_COPY) * p1 = silu(h1)*h3*SH.
            s = sp.tile([P, nsz], f32, tag="s", name=f"fs_{ph}_{m}_{n0}")
            nc.scalar.activation(s[:], p1[:],
                                 mybir.ActivationFunctionType.Sigmoid,
                                 scale=1.0 / S1)
            c3 = sp.tile([P, nsz], f32, tag="c3", name=f"fc3_{ph}_{m}_{n0}")
            nc.scalar.activation(c3[:], p3[:],
                                 mybir.ActivationFunctionType.Copy,
                                 scale=K_COPY)
            # t = sig*p1 first (not sig*c3): p1's last reader is then the
            # FIRST DVE op, so both PSUM banks free one op earlier.
            t = sp.tile([P, nsz], f32, tag="t", name=f"ft_{ph}_{m}_{n0}")
            nc.vector.tensor_mul(t[:], s[:], p1[:])
            nc.vector.tensor_mul(hq[:, m, n0:n0 + nsz], t[:], c3[:])

    yield  # phase 1 done

    # Phase-2 weights: one 3MB SP-ring DMA issued after the phase-1 panels
    # (needed only at phase 2; issuing it in the header stalls the panels).
    wf2t = w2fp.tile([P, JP8, 2, d_out], f8, tag="wf2", name=f"wf2_{ph}")
    nc.sync.dma_start(out=wf2t[:], in_=wf2_dram[:])

    # ---- phase 2: out[tok] = (comb/(SH*SW2)) * (hq.T @ W2q) ----
    ps2 = [(psY, "py"), (psY, "py"), (psY, "py"), (psY, "py"),
           (psA, "p1"), (psA, "p1"), (psA, "p3"), (psA, "p3")]
    idx = 0
    for mt2 in range(kd):
        osb = op.tile([P, cp], dt, tag="osb", name=f"fosb_{ph}_{mt2}")
        for n0 in range(0, cp, NSZ):
            nn = min(NSZ, cp - n0)
            pool, ptag = ps2[idx % len(ps2)]
            idx += 1
            py = pool.tile([P, nn], f32, tag=ptag, name=f"fpy_{ph}_{mt2}_{n0}")
            for jp in range(JP8):
                nc.tensor.matmul(py[:], wf2t[:, jp, :, mt2 * P:(mt2 + 1) * P],
                                 hq[:, 2 * jp:2 * jp + 2, n0:n0 + nn],
                                 start=(jp == 0), stop=(jp == JP8 - 1),
                                 perf_mode=mybir.MatmulPerfMode.DoubleRow)
            nc.vector.tensor_mul(osb[:, n0:n0 + nn], py[:],
                                 cft[:, n0:n0 + nn])
        nc.scalar.dma_start(out=out_rows[mt2], in_=osb[:])


def build_program(CBs, CFs, cfg):
    """Per-core Bass program.  CBs[j]/CFs[j] = bf16/fp8 token capacities of
    routed expert slot j."""
    nc = bacc.Bacc()
    dt = mybir.dt.bfloat16
    f8 = mybir.dt.float8e4
    f32 = mybir.dt.float32
    dv = _derived(cfg)
    epc, kd, it_r, it_s = dv["epc"], dv["kd"], dv["it_r"], dv["it_s"]
    sh_tok = dv["sh_tok"]
    d_out = cfg["d_out"]
    nkg = kd // WKG

    ins = {}
    for j in range(epc):
        ins[f"xt{j}"] = nc.dram_tensor(f"xt{j}", [kd // XG, P, XG, CBs[j]], dt, kind="ExternalInput")
        ins[f"w13_{j}"] = nc.dram_tensor(f"w13_{j}", [it_r, P, nkg, 2, WKG, P], dt, kind="ExternalInput")
        ins[f"w2_{j}"] = nc.dram_tensor(f"w2_{j}", [it_r, P, d_out], dt, kind="ExternalInput")
        ins[f"cb{j}"] = nc.dram_tensor(f"cb{j}", [P, CBs[j]], f32, kind="ExternalInput")
        ins[f"xf{j}"] = nc.dram_tensor(f"xf{j}", [kd // XG, P, XG, CFs[j]], f8, kind="ExternalInput")
        ins[f"wf13_{j}"] = nc.dram_tensor(f"wf13_{j}", [it_r, P, KP8, 2, 2, P], f8, kind="ExternalInput")
        ins[f"wf2_{j}"] = nc.dram_tensor(f"wf2_{j}", [P, JP8, 2, d_out], f8, kind="ExternalInput")
        ins[f"cf{j}"] = nc.dram_tensor(f"cf{j}", [P, CFs[j]], f32, kind="ExternalInput")
    ins["xts"] = nc.dram_tensor("xts", [kd // XG, P, XG, sh_tok], dt, kind="ExternalInput")
    ins["ws13"] = nc.dram_tensor("ws13", [it_s, P, nkg, 2, WKG, P], dt, kind="ExternalInput")
    ins["ws2"] = nc.dram_tensor("ws2", [it_s, P, d_out], dt, kind="ExternalInput")

    outs = {}
    for j in range(epc):
        outs[f"y{j}"] = nc.dram_tensor(f"y{j}", [kd, P, CBs[j]], dt, kind="ExternalOutput")
        outs[f"yf{j}"] = nc.dram_tensor(f"yf{j}", [kd, P, CFs[j]], dt, kind="ExternalOutput")
    outs["z"] = nc.dram_tensor("z", [kd, P, sh_tok], dt, kind="ExternalOutput")

    with tile.TileContext(nc) as tc:
        with (
            tc.tile_pool(name="xt", bufs=1) as xtp,
            tc.tile_pool(name="xf", bufs=1) as xfp,
            tc.tile_pool(name="w", bufs=3) as wp,
            tc.tile_pool(name="wf", bufs=2) as wfp,
            tc.tile_pool(name="hp", bufs=1) as hpp,
            tc.tile_pool(name="hq", bufs=2) as hqp,
            tc.tile_pool(name="w2", bufs=1) as w2p,
            tc.tile_pool(name="w2f", bufs=1) as w2fp,
            tc.tile_pool(name="s", bufs=2) as sp,
            tc.tile_pool(name="o", bufs=3) as op,
            tc.tile_pool(name="cg", bufs=1) as cgp,
            tc.tile_pool(name="warm", bufs=1) as wmp,
            tc.tile_pool(name="psA", bufs=2, space="PSUM") as psA,
            tc.tile_pool(name="psY", bufs=4, space="PSUM") as psY,
        ):
            pools = dict(xt=xtp, xf=xfp, w=wp, wf=wfp, hp=hpp, hq=hqp,
                         w2=w2p, w2f=w2fp, s=sp, o=op, cg=cgp,
                         psA=psA, psY=psY)
            # PE warm-up: dummy matmuls with no DMA dependency keep the HAM
            # activity monitor busy during the initial DMA wait.
            wt = wmp.tile([P, NSZ], dt, tag="wt", name="warm_src")
            nc.gpsimd.memset(wt[:], 0)
            wps = psY.tile([P, NSZ], f32, tag="py", name="warm_ps")
            for i in range(11):
                nc.tensor.matmul(wps[:], wt[:, :P], wt[:],
                                 start=True, stop=True)
            # Phase order: both bf16 phases first (their large weight DMAs
            # overlap the large bf16 compute), then the fp8 phases, then the
            # shared expert.
            # Per-phase alternating A/B buffer tags (bufs=1) make every
            # phase's input DMAs first-use (no buffer-recycle waits), so the
            # SP ring streams the whole program's inputs in demand order.
            # Phase interleave bf/f8 + software-pipelined emission: each
            # phase's DMA header is emitted right after the PREVIOUS phase's
            # phase-1 panels, so on the serial SP ring it transfers during
            # that phase's compute instead of queuing behind its w2 bulk.
            gens = []
            for j in range(epc):
                gens.append(_emit_phase(
                    nc, pools, ins[f"xt{j}"], ins[f"w13_{j}"],
                    ins[f"w2_{j}"], ins[f"cb{j}"],
                    [outs[f"y{j}"][mt2] for mt2 in range(kd)],
                    it_r, CBs[j], cfg, ph=f"e{j}", grp="AB"[j % 2],
                    first=(j == 0)))
                gens.append(_emit_phase_f8(
                    nc, pools, ins[f"xf{j}"], ins[f"wf13_{j}"],
                    ins[f"wf2_{j}"], ins[f"cf{j}"],
                    [outs[f"yf{j}"][mt2] for mt2 in range(kd)],
                    it_r, CFs[j], cfg, ph=f"f{j}", grp="AB"[j % 2]))
            gens.append(_emit_phase(
                nc, pools, ins["xts"], ins["ws13"], ins["ws2"], None,
                [outs["z"][mt2] for mt2 in range(kd)],
                it_s, sh_tok, cfg, ph="s", grp="A"))

            def step(g):
                next(g, None)

            # (Measured dead end: hoisting headers 2 phases early produced
            # NaN output — a timing-exposed race — with no gap improvement.)
            step(gens[0])          # hdr p0
            step(gens[0])          # ph1 p0
            for i in range(1, len(gens)):
                step(gens[i])      # hdr p_i
                step(gens[i - 1])  # ph2 p_{i-1}
                step(gens[i])      # ph1 p_i
            step(gens[-1])         # ph2 p_last
    nc.compile()
    return nc


def _panelize_w13(w1, w3, itiles):
    """(D, I) x2 -> (itiles, P, kd//WKG, 2, WKG, P) bf16 panels."""
    dd, ii = w1.shape
    kd = dd // P
    p1 = w1.reshape(kd, P, itiles, P).transpose(2, 1, 0, 3)
    p3 = w3.reshape(kd, P, itiles, P).transpose(2, 1, 0, 3)
    panel = np.stack([p1, p3], axis=2)           # (it, P, 2, kd, P)
    panel = panel.reshape(itiles, P, 2, kd // WKG, WKG, P)
    return np.ascontiguousarray(panel.transpose(0, 1, 3, 2, 4, 5))


def _q8(a, s):
    return np.clip(a * s, -240.0, 240.0).astype(F8)


def _panelize_w13_f8(w1q, w3q, itiles):
    """e4m3 (D, I) x2 -> (itiles, P, KP8, 2{w1,w3}, 2{pair}, P)."""
    pan = np.stack([w1q.reshape(KP8, 2, P, itiles, P),
                    w3q.reshape(KP8, 2, P, itiles, P)], axis=0)
    # (w, kp, s, p, m, i) -> (m, p, kp, w, s, i)
    return np.ascontiguousarray(pan.transpose(4, 3, 1, 0, 2, 5))


def _pack_w2_f8(w2q):
    """e4m3 (I, D) -> (P, JP8, 2, D) with I zero-padded to 2*JP8*P rows."""
    ii, dd = w2q.shape
    pad = np.zeros((2 * JP8 * P, dd), F8)
    pad[:ii] = w2q
    return np.ascontiguousarray(
        pad.reshape(JP8, 2, P, dd).transpose(2, 0, 1, 3))


def prep(x, weights, indices, W1, W3, W2, Ws1, Ws3, Ws2, cfg):
    """Host-side dispatch: tier split, shard/gather/pad/cast/pre-tile."""
    T, D, E = cfg["T"], cfg["D"], cfg["E"]
    dv = _derived(cfg)
    epc, kd, it_r, it_s = dv["epc"], dv["kd"], dv["it_r"], dv["it_s"]
    nt, sh_tok = dv["n_tok_shards"], dv["sh_tok"]
    sh_half = cfg["sh_half"]

    xf = np.asarray(x, F32).reshape(T, D)
    wts = np.asarray(weights, F32)
    idx = np.asarray(indices).astype(np.int64)
    W1 = np.asarray(W1, F32)
    W3 = np.asarray(W3, F32)
    W2 = np.asarray(W2, F32)
    Ws1 = np.asarray(Ws1, F32)
    Ws3 = np.asarray(Ws3, F32)
    Ws2 = np.asarray(Ws2, F32)

    # Per-(token, expert) combine weight; duplicate expert ids accumulate.
    comb = np.zeros((T, E), F32)
    np.add.at(comb, (np.arange(T)[:, None], idx), wts)

    # Tier split per expert: bf16 tokens (comb >= T_FP8) and fp8 tokens.
    tok_bf = [np.nonzero(comb[:, e] >= T_FP8)[0] for e in range(E)]
    tok_f8 = [np.nonzero((comb[:, e] > 0) & (comb[:, e] < T_FP8))[0]
              for e in range(E)]
    nb = np.array([len(t) for t in tok_bf])
    nf = np.array([len(t) for t in tok_f8])

    # Exact 2-slot assignment: minimize CB0+CB1 + w*(CF0+CF1) over all
    # 16-choose-8 splits (w ~ fp8 column cost relative to bf16).
    import itertools
    best = None
    for s0 in itertools.combinations(range(E), epc * cfg["ncores"] // 2):
        s0 = list(s0)
        s1 = [i for i in range(E) if i not in s0]
        cost = (nb[s0].max() + nb[s1].max()
                + 0.62 * (nf[s0].max() + nf[s1].max()))
        if best is None or cost < best[0]:
            best = (cost, s0, s1)
    eslot = np.array([best[1], best[2]])  # eslot[j][c] = expert id

    CBs = [max(64, -(-int(nb[eslot[j]].max()) // 4) * 4) for j in range(epc)]
    CFs = [max(64, -(-int(nf[eslot[j]].max()) // 16) * 16) for j in range(epc)]

    xT = np.ascontiguousarray(xf.T)                 # (D, T) fp32
    xT16 = xT.astype(BF16)
    xTq = _q8(xT, SX)

    def _xt_layout(cols):
        # (D, n) -> (kd//XG, P, XG, n): one contiguous DMA per k-tile group.
        n = cols.shape[1]
        return np.ascontiguousarray(
            cols.reshape(kd // XG, XG, P, n).swapaxes(1, 2))

    in_maps = []
    for c in range(cfg["ncores"]):
        m = {}
        for j in range(epc):
            e = int(eslot[j][c])
            # bf16 tier
            toks = tok_bf[e]
            tpad = np.zeros(CBs[j], np.int64)
            tpad[:len(toks)] = toks
            m[f"xt{j}"] = _xt_layout(xT16[:, tpad])
            m[f"w13_{j}"] = _panelize_w13(W1[e], W3[e], it_r).astype(BF16)
            m[f"w2_{j}"] = np.ascontiguousarray(
                W2[e].reshape(it_r, P, cfg["d_out"])).astype(BF16)
            cg = np.zeros(CBs[j], F32)
            cg[:len(toks)] = comb[toks, e]
            m[f"cb{j}"] = np.ascontiguousarray(np.broadcast_to(cg, (P, CBs[j])))
            # fp8 tier
            toksf = tok_f8[e]
            tpadf = np.zeros(CFs[j], np.int64)
            tpadf[:len(toksf)] = toksf
            m[f"xf{j}"] = _xt_layout(xTq[:, tpadf])
            m[f"wf13_{j}"] = _panelize_w13_f8(_q8(W1[e], SW), _q8(W3[e], SW), it_r)
            m[f"wf2_{j}"] = _pack_w2_f8(_q8(W2[e], SW2))
            cgf = np.zeros(CFs[j], F32)
            cgf[:len(toksf)] = comb[toksf, e] / (SH * SW2)
            m[f"cf{j}"] = np.ascontiguousarray(np.broadcast_to(cgf, (P, CFs[j])))
        # Shared expert: 2-way inter split x (ncores/2)-way token split.
        h, q = divmod(c, nt)
        m["xts"] = _xt_layout(xT16[:, q * sh_tok:(q + 1) * sh_tok])
        m["ws13"] = _panelize_w13(Ws1[:, h * sh_half:(h + 1) * sh_half],
                                  Ws3[:, h * sh_half:(h + 1) * sh_half],
                                  it_s).astype(BF16)
        m["ws2"] = np.ascontiguousarray(
            Ws2[h * sh_half:(h + 1) * sh_half].reshape(it_s, P, cfg["d_out"])).astype(BF16)
        in_maps.append(m)

    meta = dict(tok_bf=tok_bf, tok_f8=tok_f8, nb=nb, nf=nf,
                CBs=CBs, CFs=CFs, eslot=eslot)
    return in_maps, meta


def combine(results, meta, cfg):
    """Host-side unshard: sum shared partials, scatter-add routed outputs."""
    T, D = cfg["T"], cfg["d_out"]
    dv = _derived(cfg)
    epc, nt, sh_tok = dv["epc"], dv["n_tok_shards"], dv["sh_tok"]
    out = np.zeros((T, D), F32)
    for c in range(cfg["ncores"]):
        r = results[c]
        q = c % nt
        out[q * sh_tok:(q + 1) * sh_tok] += \
            np.asarray(r["z"], F32).reshape(D, sh_tok).T
        for j in range(epc):
            e = int(meta["eslot"][j][c])
            yt = np.asarray(r[f"y{j}"], F32).reshape(D, -1)
            out[meta["tok_bf"][e]] += yt.T[:meta["nb"][e]]
            yf = np.asarray(r[f"yf{j}"], F32).reshape(D, -1)
            out[meta["tok_f8"][e]] += yf.T[:meta["nf"][e]]
    return out


# Test-harness knobs (kernel() callers get no-trace defaults).
TRACE = False
TMPDIR = None
LAST_RESULT = None


def kernel(x, weights, indices, W1, W3, W2, Ws1, Ws3, Ws2):
    global LAST_RESULT
    cfg = FULL_CFG
    in_maps, meta = prep(x, weights, indices, W1, W3, W2,
                         Ws1, Ws3, Ws2, cfg)
    nc = build_program(meta["CBs"], meta["CFs"], cfg)
    res = run_bass_kernel_spmd(nc, in_maps, core_ids=list(range(cfg["ncores"])),
                               trace=TRACE, tmpdir=TMPDIR)
    LAST_RESULT = res
    out = combine(res.results, meta, cfg)
    return out.reshape(1, cfg["T"], cfg["D"]).astype(F32)


# revision 63
# speedup vs baseline: 1.0416x; 1.0416x over previous
"""MoE (DeepSeek-style, no gate) SwiGLU kernel for 8 Trainium2 NeuronCores.

Strategy (expert parallelism, per the sharding hint):
  - 16 routed experts sharded 2-per-core across 8 cores.
  - Token dispatch done host-side: for each expert, gather the tokens routed
    to it (topk membership), pad to a common capacity, and ship the
    pre-transposed activation columns xT[:, tokens] to the owning core.
  - Shared expert tensor-parallel over its inter dim (2816/2 halves) x
    token quarters, computed on all 2048 tokens in bf16.
  - Each core runs the same Bass program (SPMD) on its own shard; the host
    scatter-adds routed outputs and sums shared-expert partials.

Two precision tiers for routed tokens (tier by combine weight):
  - comb >= T_FP8: bf16 pipeline (PE at 1 col/cycle).  The output error
    contribution of a routed pair scales with comb, so large-comb pairs
    need bf16.
  - comb <  T_FP8: fp8 e4m3 pipeline with perf_mode=DoubleRow (2 k-tiles
    per matmul, ~1.4x PE throughput).  All quantization scales are powers
    of two folded into ACT scale operands and the host-side combine
    weights, so the fp8 path has the same instruction structure as bf16.
  Measured (numpy sim of e4m3-quantized pipeline on the real inputs):
  T_FP8=0.45 -> rel err ~1.4e-2 vs the 2e-2 gate; all-bf16 is 3.9e-3.

Compute layout per phase (one phase = one SwiGLU MLP on a token set):
  phase 1:  h1T = W1p.T @ xT, h3T = W3p.T @ xT   (I on partitions, tokens free)
            h' = silu(h1T) * h3T                  -> bf16 (or e4m3) in SBUF
  phase 2:  y[tok_tile] = h'.T @ W2p, scaled per-token by comb weight on
            PSUM eviction (DVE mul with a broadcast comb row).

Overlap structure (each item measured against the 437us all-bf16 baseline;
this version: ~430us, rel err 1.46e-2):
  - Phase interleave e0,f0,e1,f1,shared: big bf16 weight prefetches overlap
    an fp8 phase's compute in addition to a phase 2, and buffer-tag
    rotations (w13/wf13) get a whole phase of slack before reuse.
  - Per-phase alternating A/B tags (bufs=1) for xt/xf/cb: every phase's
    input DMAs are first-use, so the serial SP HWDGE ring streams them in
    demand order instead of head-of-line blocking on buffer-recycle waits.
  - Software-pipelined emission (generators): each phase's DMA header is
    emitted right after the previous phase's phase-1 panels, ahead of its
    w2 bulk on the ring.
  - Output DMAs ride the ACT ring: on the SP ring they pace the next
    phase's inputs to phase-2 compute.
  - Ramp: 11 dummy warm-up matmuls at t=0 keep the PE HAM activity monitor
    busy during the initial DMA wait; the first phase's m=0 W13 panel is
    split into 4 k-group DMAs interleaved with the xt token groups in
    SP-ring order, first two xt groups on the then-idle ACT ring.
Measured dead ends: fp8 for the shared expert or large-comb pairs fails the
2e-2 gate; dropping small-comb pairs outright fails (max-abs metric); int8
matmul unsupported by bass; >2 ACT-ring ramp DMAs stall the PE; w2/wf2 on
the gpsimd SWDGE ring (pushes at t=0 steal ramp HBM bw) or the ACT ring
(gated behind ph1 ACTIVATEs, arrive too late); psA=3/psY=2 PSUM split;
hoisting headers 2 phases early (NaN race); routing later phases' m=0/m=1
through the per-kg ramp tags (scheduler gates w2 pushes on late matmul
counters).
"""

import numpy as np
import ml_dtypes

import concourse.bass as bass
import concourse.bacc as bacc
import concourse.mybir as mybir
import concourse.tile as tile
from concourse.bass_utils import run_bass_kernel_spmd

BF16 = ml_dtypes.bfloat16
F8 = ml_dtypes.float8_e4m3
F32 = np.float32
P = 128
NSZ = 512   # PSUM bank free size (fp32)
XG = 2      # xt k-tiles per DMA group (= one DoubleRow pair)
WKG = 4     # w13 k-tiles per k-group block (bf16 path)

# fp8 tier: threshold and power-of-two quantization scales.
T_FP8 = 0.45
SX = 32.0       # x -> e4m3 scale
SW = 1024.0     # W1/W3 -> e4m3 scale
SH = 8.0        # h' -> e4m3 scale
SW2 = 1024.0    # W2 -> e4m3 scale
S1 = SX * SW                  # PSUM p1/p3 carry h*S1
K_COPY = SH / (S1 * S1)       # ACT copy scale: c3 = p3 * K_COPY
KP8 = 8                       # phase-1 k-tile pairs (D/128/2)
JP8 = 6                       # phase-2 I-tile pairs (11 -> pad to 12)

FULL_CFG = dict(
    ncores=8,
    T=2048,
    D=2048,
    E=16,
    I_E=1408,
    sh_half=1408,    # shared expert sharded 2 (inter) x ncores/2 (tokens)
    d_out=2048,
)


def _derived(cfg):
    nt = max(1, cfg["ncores"] // 2)
    return dict(
        epc=cfg["E"] // cfg["ncores"],
        kd=cfg["D"] // P,
        it_r=cfg["I_E"] // P,
        it_s=cfg["sh_half"] // P,
        n_tok_shards=nt,
        sh_tok=cfg["T"] // nt,
    )


def _emit_phase(nc, pools, xt_dram, w13_dram, w2_dram, cb_dram,
                out_rows, n_itiles, cp, cfg, ph, grp, first=False):
    """One bf16 SwiGLU MLP phase over `cp` token columns, `n_itiles` I-tiles.

    Output is TRANSPOSED: out_rows[mt2] is the DRAM destination for model-dim
    tile mt2 ([128, cp] = [D-tile, tokens]).  cb_dram is None for the shared
    expert; otherwise the combine weight broadcast to [128, cp].  `first`
    marks the program's first phase: its m=0 W13 panel DMA is split per
    k-group and interleaved with the xt groups for the ramp.
    """
    kd = _derived(cfg)["kd"]
    d_out = cfg["d_out"]
    dt = mybir.dt.bfloat16
    f32 = mybir.dt.float32
    nkg = kd // WKG

    xtp, wp, hpp, w2p, sp, op, cgp, psA, psY = (
        pools["xt"], pools["w"], pools["hp"], pools["w2"], pools["s"],
        pools["o"], pools["cg"], pools["psA"], pools["psY"])

    # DMA header. The SP HWDGE ring executes DMAs serially in issue order,
    # so for the program's first phase the m=0 panel k-groups and the xt
    # token-k groups are interleaved to match the matmul consumption order.
    xtg = []
    if first:
        wkg = [wp.tile([P, 2, WKG, P], dt, tag=f"w13a{kg}", bufs=2,
                       name=f"w13a_{ph}_{kg}") for kg in range(nkg)]
        wpre = [wkg]
        for g in range(kd // XG):
            xtg.append(xtp.tile([P, XG, cp], dt, tag=f"xt{grp}_{g}",
                                name=f"xt_{ph}_{g}"))
        # First two xt groups ride the otherwise-idle ACT HWDGE ring.
        nc.scalar.dma_start(out=xtg[0][:], in_=xt_dram[0])
        nc.scalar.dma_start(out=xtg[1][:], in_=xt_dram[1])
        # SP ring in PE demand order, weights one k-group ahead.
        for kind, i in (("w", 0), ("w", 1), ("x", 2), ("x", 3), ("w", 2),
                        ("x", 4), ("x", 5), ("w", 3), ("x", 6), ("x", 7)):
            if kind == "w":
                nc.sync.dma_start(out=wkg[i][:], in_=w13_dram[0][:, i])
            else:
                nc.sync.dma_start(out=xtg[i][:], in_=xt_dram[i])
    else:
        wpre = [wp.tile([P, nkg, 2, WKG, P], dt, tag="w13", name=f"w13_{ph}_0")]
        nc.sync.dma_start(out=wpre[0][:], in_=w13_dram[0])
        for g in range(kd // XG):
            xge = xtp.tile([P, XG, cp], dt, tag=f"xt{grp}_{g}",
                           name=f"xt_{ph}_{g}")
            nc.sync.dma_start(out=xge[:], in_=xt_dram[g])
            xtg.append(xge)
    if n_itiles > 1:
        # (Measured dead end: routing phases' m=0/m=1 through per-kg ramp
        # tags made the scheduler gate this phase's w2 pushes on late-phase
        # matmul counters -> 12us stall mid phase-2.)
        w13b = wp.tile([P, nkg, 2, WKG, P], dt, tag="w13", name=f"w13_{ph}_1")
        nc.sync.dma_start(out=w13b[:], in_=w13_dram[1])
        wpre.append(w13b)

    cbt = None
    if cb_dram is not None:
        cbr = cgp.tile([P, cp], f32, tag=f"cbr{grp}", name=f"cbr_{ph}")
        nc.sync.dma_start(out=cbr[:], in_=cb_dram[:])
        # Bounce through DVE so the per-tile eviction muls below need only
        # the PE wait (DVE has already observed the cb DMA here).
        cbt = cgp.tile([P, cp], f32, tag=f"cb{grp}", name=f"cb_{ph}")
        nc.vector.tensor_copy(cbt[:], cbr[:])

    yield  # header done (emitted one phase early so the ring prefetches it)

    # ---- phase 1: h' = silu(xW1) * (xW3), transposed layout [I, tokens] ----
    hp = []
    for m in range(n_itiles):
        if m < len(wpre):
            w13t = wpre[m]
        else:
            w13t = wp.tile([P, nkg, 2, WKG, P], dt, tag="w13", name=f"w13_{ph}_{m}")
            nc.sync.dma_start(out=w13t[:], in_=w13_dram[m])
        hpm = hpp.tile([P, cp], dt, tag=f"hp_{m}", name=f"hp_{ph}_{m}")
        for n0 in range(0, cp, NSZ):
            nsz = min(NSZ, cp - n0)
            p1 = psA.tile([P, nsz], f32, tag="p1", name=f"p1_{ph}_{m}_{n0}")
            p3 = psA.tile([P, nsz], f32, tag="p3", name=f"p3_{ph}_{m}_{n0}")
            for kt in range(kd):
                xs = xtg[kt // XG][:, kt % XG, n0:n0 + nsz]
                if isinstance(w13t, list):   # first phase m=0/1: per-kg tiles
                    w1s = w13t[kt // WKG][:, 0, kt % WKG, :]
                    w3s = w13t[kt // WKG][:, 1, kt % WKG, :]
                else:
                    w1s = w13t[:, kt // WKG, 0, kt % WKG, :]
                    w3s = w13t[:, kt // WKG, 1, kt % WKG, :]
                nc.tensor.matmul(p1[:], w1s, xs,
                                 start=(kt == 0), stop=(kt == kd - 1))
                nc.tensor.matmul(p3[:], w3s, xs,
                                 start=(kt == 0), stop=(kt == kd - 1))
            # silu(h1)*h3 = sigmoid(h1)*h3*h1.
            s = sp.tile([P, nsz], f32, tag="s", name=f"s_{ph}_{m}_{n0}")
            nc.scalar.activation(s[:], p1[:],
                                 mybir.ActivationFunctionType.Sigmoid)
            c3 = sp.tile([P, nsz], f32, tag="c3", name=f"c3_{ph}_{m}_{n0}")
            nc.scalar.copy(c3[:], p3[:])
            t = sp.tile([P, nsz], f32, tag="t", name=f"t_{ph}_{m}_{n0}")
            nc.vector.tensor_mul(t[:], s[:], c3[:])
            nc.vector.tensor_mul(hpm[:, n0:n0 + nsz], t[:], p1[:])
        hp.append(hpm)

    yield  # phase 1 done

    # ---- phase 2: out[tok] = comb * (h'.T @ W2) ----
    # Phase-2 weights on the SP ring.  (Measured dead ends: gpsimd/SWDGE
    # pushes jump the queue at t=0 and steal HBM bandwidth from the ramp;
    # ACT-ring pushes are gated behind phase-1 ACTIVATEs and arrive too
    # late to prefetch.)
    w2t = []
    for kt in range(n_itiles):
        w = w2p.tile([P, d_out], dt, tag=f"w2_{kt}", name=f"w2_{ph}_{kt}")
        nc.sync.dma_start(out=w[:], in_=w2_dram[kt])
        w2t.append(w)

    ps2 = [(psY, "py"), (psY, "py"), (psY, "py"), (psY, "py"),
           (psA, "p1"), (psA, "p1"), (psA, "p3"), (psA, "p3")]
    idx = 0
    for mt2 in range(kd):
        osb = op.tile([P, cp], dt, tag="osb", name=f"osb_{ph}_{mt2}")
        for n0 in range(0, cp, NSZ):
            nn = min(NSZ, cp - n0)
            pool, ptag = ps2[idx % len(ps2)]
            idx += 1
            py = pool.tile([P, nn], f32, tag=ptag, name=f"py_{ph}_{mt2}_{n0}")
            for kt in range(n_itiles):
                nc.tensor.matmul(py[:], w2t[kt][:, mt2 * P:(mt2 + 1) * P],
                                 hp[kt][:, n0:n0 + nn],
                                 start=(kt == 0), stop=(kt == n_itiles - 1))
            if cbt is not None:
                nc.vector.tensor_mul(osb[:, n0:n0 + nn], py[:],
                                     cbt[:, n0:n0 + nn])
            elif idx % 2:
                nc.vector.tensor_copy(osb[:, n0:n0 + nn], py[:])
            else:
                nc.scalar.copy(osb[:, n0:n0 + nn], py[:])
        # Output DMAs ride the ACT ring: they are gated on eviction anyway,
        # and on the SP ring they pace the next phase's inputs to phase-2
        # compute (head-of-line blocking).
        nc.scalar.dma_start(out=out_rows[mt2], in_=osb[:])


def _emit_phase_f8(nc, pools, xf_dram, wf13_dram, wf2_dram, cf_dram,
                   out_rows, n_itiles, cp, cfg, ph, grp):
    """fp8 e4m3 DoubleRow SwiGLU phase over `cp` token columns.

    Same structure as the bf16 phase but every matmul is a DoubleRow pair
    (2 k-tiles per instruction).  PSUM p1/p3 carry h*S1; the quantization
    scales are folded into the ACT scale operands (phase 1) and into the
    host-prepared combine row cf = comb/(SH*SW2) (phase 2).  Phase 2 pads
    the 11 I-tiles to 12 with a zero 12th h' tile (and zero W2 rows).
    """
    kd = _derived(cfg)["kd"]
    d_out = cfg["d_out"]
    f8 = mybir.dt.float8e4
    dt = mybir.dt.bfloat16
    f32 = mybir.dt.float32

    xfp, wfp, hqp, w2fp, sp, op, cgp, psA, psY = (
        pools["xf"], pools["wf"], pools["hq"], pools["w2f"], pools["s"],
        pools["o"], pools["cg"], pools["psA"], pools["psY"])

    # DMA header: xt pair-groups + first two weight panels.
    xfg = []
    for g in range(kd // XG):
        xge = xfp.tile([P, XG, cp], f8, tag=f"xf{grp}_{g}", name=f"xf_{ph}_{g}")
        nc.sync.dma_start(out=xge[:], in_=xf_dram[g])
        xfg.append(xge)
    wpre = []
    for m in range(min(3, n_itiles)):
        wt = wfp.tile([P, KP8, 2, 2, P], f8, tag="wf13", bufs=3,
                      name=f"wf13_{ph}_{m}")
        nc.sync.dma_start(out=wt[:], in_=wf13_dram[m])
        wpre.append(wt)

    cfr = cgp.tile([P, cp], f32, tag=f"cfr{grp}", name=f"cfr_{ph}")
    nc.sync.dma_start(out=cfr[:], in_=cf_dram[:])
    cft = cgp.tile([P, cp], f32, tag=f"cf{grp}", name=f"cf_{ph}")
    nc.vector.tensor_copy(cft[:], cfr[:])

    yield  # header done

    # ---- phase 1: hq = e4m3(silu(h1) * h3 * SH), layout [I-tile, tokens] ----
    hq = hqp.tile([P, JP8 * 2, cp], f8, tag="hq", name=f"hq_{ph}")
    nc.gpsimd.memset(hq[:, n_itiles:, :], 0)   # zero-pad I-tiles 11..
    for m in range(n_itiles):
        if m < len(wpre):
            w13t = wpre[m]
        else:
            w13t = wfp.tile([P, KP8, 2, 2, P], f8, tag="wf13", bufs=3,
                            name=f"wf13_{ph}_{m}")
            nc.sync.dma_start(out=w13t[:], in_=wf13_dram[m])
        for n0 in range(0, cp, NSZ):
            nsz = min(NSZ, cp - n0)
            p1 = psA.tile([P, nsz], f32, tag="p1", name=f"q1_{ph}_{m}_{n0}")
            p3 = psA.tile([P, nsz], f32, tag="p3", name=f"q3_{ph}_{m}_{n0}")
            for kp in range(KP8):
                xs = xfg[kp][:, :, n0:n0 + nsz]
                nc.tensor.matmul(p1[:], w13t[:, kp, 0], xs,
                                 start=(kp == 0), stop=(kp == KP8 - 1),
                                 perf_mode=mybir.MatmulPerfMode.DoubleRow)
                nc.tensor.matmul(p3[:], w13t[:, kp, 1], xs,
                                 start=(kp == 0), stop=(kp == KP8 - 1),
                                 perf_mode=mybir.MatmulPerfMode.DoubleRow)
            # hq = sigmoid(p1/S1) * (p3*K_COPY) * p1 = silu(h1)*h3*SH.
            s = sp.tile([P, nsz], f32, tag="s", name=f"fs_{ph}_{m}_{n0}")
            nc.scalar.activation(s[:], p1[:],
                                 mybir.ActivationFunctionType.Sigmoid,
                                 scale=1.0 / S1)
            c3 = sp.tile([P, nsz], f32, tag="c3", name=f"fc3_{ph}_{m}_{n0}")
            nc.scalar.activation(c3[:], p3[:],
                                 mybir.ActivationFunctionType.Copy,
                                 scale=K_COPY)
            # (Measured dead ends: odd-m accumulators on the psY banks, and
            # t=s*p1-first mul order — both regressed ~15us via scheduler
            # cross-tag serialization, like the w13a-tag reuse attempt.)
            t = sp.tile([P, nsz], f32, tag="t", name=f"ft_{ph}_{m}_{n0}")
            nc.vector.tensor_mul(t[:], s[:], c3[:])
            nc.vector.tensor_mul(hq[:, m, n0:n0 + nsz], t[:], p1[:])

    yield  # phase 1 done

    # Phase-2 weights: one 3MB SP-ring DMA issued after the phase-1 panels
    # (needed only at phase 2; issuing it in the header stalls the panels).
    wf2t = w2fp.tile([P, JP8, 2, d_out], f8, tag="wf2", name=f"wf2_{ph}")
    nc.sync.dma_start(out=wf2t[:], in_=wf2_dram[:])

    # ---- phase 2: out[tok] = (comb/(SH*SW2)) * (hq.T @ W2q) ----
    ps2 = [(psY, "py"), (psY, "py"), (psY, "py"), (psY, "py"),
           (psA, "p1"), (psA, "p1"), (psA, "p3"), (psA, "p3")]
    idx = 0
    for mt2 in range(kd):
        osb = op.tile([P, cp], dt, tag="osb", name=f"fosb_{ph}_{mt2}")
        for n0 in range(0, cp, NSZ):
            nn = min(NSZ, cp - n0)
            pool, ptag = ps2[idx % len(ps2)]
            idx += 1
            py = pool.tile([P, nn], f32, tag=ptag, name=f"fpy_{ph}_{mt2}_{n0}")
            for jp in range(JP8):
                nc.tensor.matmul(py[:], wf2t[:, jp, :, mt2 * P:(mt2 + 1) * P],
                                 hq[:, 2 * jp:2 * jp + 2, n0:n0 + nn],
                                 start=(jp == 0), stop=(jp == JP8 - 1),
                                 perf_mode=mybir.MatmulPerfMode.DoubleRow)
            nc.vector.tensor_mul(osb[:, n0:n0 + nn], py[:],
                                 cft[:, n0:n0 + nn])
        nc.scalar.dma_start(out=out_rows[mt2], in_=osb[:])


def build_program(CBs, CFs, cfg):
    """Per-core Bass program.  CBs[j]/CFs[j] = bf16/fp8 token capacities of
    routed expert slot j."""
    nc = bacc.Bacc()
    dt = mybir.dt.bfloat16
    f8 = mybir.dt.float8e4
    f32 = mybir.dt.float32
    dv = _derived(cfg)
    epc, kd, it_r, it_s = dv["epc"], dv["kd"], dv["it_r"], dv["it_s"]
    sh_tok = dv["sh_tok"]
    d_out = cfg["d_out"]
    nkg = kd // WKG

    ins = {}
    for j in range(epc):
        ins[f"xt{j}"] = nc.dram_tensor(f"xt{j}", [kd // XG, P, XG, CBs[j]], dt, kind="ExternalInput")
        ins[f"w13_{j}"] = nc.dram_tensor(f"w13_{j}", [it_r, P, nkg, 2, WKG, P], dt, kind="ExternalInput")
        ins[f"w2_{j}"] = nc.dram_tensor(f"w2_{j}", [it_r, P, d_out], dt, kind="ExternalInput")
        ins[f"cb{j}"] = nc.dram_tensor(f"cb{j}", [P, CBs[j]], f32, kind="ExternalInput")
        ins[f"xf{j}"] = nc.dram_tensor(f"xf{j}", [kd // XG, P, XG, CFs[j]], f8, kind="ExternalInput")
        ins[f"wf13_{j}"] = nc.dram_tensor(f"wf13_{j}", [it_r, P, KP8, 2, 2, P], f8, kind="ExternalInput")
        ins[f"wf2_{j}"] = nc.dram_tensor(f"wf2_{j}", [P, JP8, 2, d_out], f8, kind="ExternalInput")
        ins[f"cf{j}"] = nc.dram_tensor(f"cf{j}", [P, CFs[j]], f32, kind="ExternalInput")
    ins["xts"] = nc.dram_tensor("xts", [kd // XG, P, XG, sh_tok], dt, kind="ExternalInput")
    ins["ws13"] = nc.dram_tensor("ws13", [it_s, P, nkg, 2, WKG, P], dt, kind="ExternalInput")
    ins["ws2"] = nc.dram_tensor("ws2", [it_s, P, d_out], dt, kind="ExternalInput")

    outs = {}
    for j in range(epc):
        outs[f"y{j}"] = nc.dram_tensor(f"y{j}", [kd, P, CBs[j]], dt, kind="ExternalOutput")
        outs[f"yf{j}"] = nc.dram_tensor(f"yf{j}", [kd, P, CFs[j]], dt, kind="ExternalOutput")
    outs["z"] = nc.dram_tensor("z", [kd, P, sh_tok], dt, kind="ExternalOutput")

    with tile.TileContext(nc) as tc:
        with (
            tc.tile_pool(name="xt", bufs=1) as xtp,
            tc.tile_pool(name="xf", bufs=1) as xfp,
            tc.tile_pool(name="w", bufs=3) as wp,
            tc.tile_pool(name="wf", bufs=2) as wfp,
            tc.tile_pool(name="hp", bufs=1) as hpp,
            tc.tile_pool(name="hq", bufs=2) as hqp,
            tc.tile_pool(name="w2", bufs=1) as w2p,
            tc.tile_pool(name="w2f", bufs=1) as w2fp,
            tc.tile_pool(name="s", bufs=2) as sp,
            tc.tile_pool(name="o", bufs=3) as op,
            tc.tile_pool(name="cg", bufs=1) as cgp,
            tc.tile_pool(name="warm", bufs=1) as wmp,
            tc.tile_pool(name="psA", bufs=2, space="PSUM") as psA,
            tc.tile_pool(name="psY", bufs=4, space="PSUM") as psY,
        ):
            pools = dict(xt=xtp, xf=xfp, w=wp, wf=wfp, hp=hpp, hq=hqp,
                         w2=w2p, w2f=w2fp, s=sp, o=op, cg=cgp,
                         psA=psA, psY=psY)
            # PE warm-up: dummy matmuls with no DMA dependency keep the HAM
            # activity monitor busy during the initial DMA wait.
            wt = wmp.tile([P, NSZ], dt, tag="wt", name="warm_src")
            nc.gpsimd.memset(wt[:], 0)
            wps = psY.tile([P, NSZ], f32, tag="py", name="warm_ps")
            for i in range(11):
                nc.tensor.matmul(wps[:], wt[:, :P], wt[:],
                                 start=True, stop=True)
            # Phase order: both bf16 phases first (their large weight DMAs
            # overlap the large bf16 compute), then the fp8 phases, then the
            # shared expert.
            # Per-phase alternating A/B buffer tags (bufs=1) make every
            # phase's input DMAs first-use (no buffer-recycle waits), so the
            # SP ring streams the whole program's inputs in demand order.
            # Phase interleave bf/f8 + software-pipelined emission: each
            # phase's DMA header is emitted right after the PREVIOUS phase's
            # phase-1 panels, so on the serial SP ring it transfers during
            # that phase's compute instead of queuing behind its w2 bulk.
            gens = []
            for j in range(epc):
                gens.append(_emit_phase(
                    nc, pools, ins[f"xt{j}"], ins[f"w13_{j}"],
                    ins[f"w2_{j}"], ins[f"cb{j}"],
                    [outs[f"y{j}"][mt2] for mt2 in range(kd)],
                    it_r, CBs[j], cfg, ph=f"e{j}", grp="AB"[j % 2],
                    first=(j == 0)))
                gens.append(_emit_phase_f8(
                    nc, pools, ins[f"xf{j}"], ins[f"wf13_{j}"],
                    ins[f"wf2_{j}"], ins[f"cf{j}"],
                    [outs[f"yf{j}"][mt2] for mt2 in range(kd)],
                    it_r, CFs[j], cfg, ph=f"f{j}", grp="AB"[j % 2]))
            gens.append(_emit_phase(
                nc, pools, ins["xts"], ins["ws13"], ins["ws2"], None,
                [outs["z"][mt2] for mt2 in range(kd)],
                it_s, sh_tok, cfg, ph="s", grp="A"))

            def step(g):
                next(g, None)

            # (Measured dead end: hoisting headers 2 phases early produced
            # NaN output — a timing-exposed race — with no gap improvement.)
            step(gens[0])          # hdr p0
            step(gens[0])          # ph1 p0
            for i in range(1, len(gens)):
                step(gens[i])      # hdr p_i
                step(gens[i - 1])  # ph2 p_{i-1}
                step(gens[i])      # ph1 p_i
            step(gens[-1])         # ph2 p_last
    nc.compile()
    return nc


def _panelize_w13(w1, w3, itiles):
    """(D, I) x2 -> (itiles, P, kd//WKG, 2, WKG, P) bf16 panels."""
    dd, ii = w1.shape
    kd = dd // P
    p1 = w1.reshape(kd, P, itiles, P).transpose(2, 1, 0, 3)
    p3 = w3.reshape(kd, P, itiles, P).transpose(2, 1, 0, 3)
    panel = np.stack([p1, p3], axis=2)           # (it, P, 2, kd, P)
    panel = panel.reshape(itiles, P, 2, kd // WKG, WKG, P)
    return np.ascontiguousarray(panel.transpose(0, 1, 3, 2, 4, 5))


def _q8(a, s):
    return np.clip(a * s, -240.0, 240.0).astype(F8)


def _panelize_w13_f8(w1q, w3q, itiles):
    """e4m3 (D, I) x2 -> (itiles, P, KP8, 2{w1,w3}, 2{pair}, P)."""
    pan = np.stack([w1q.reshape(KP8, 2, P, itiles, P),
                    w3q.reshape(KP8, 2, P, itiles, P)], axis=0)
    # (w, kp, s, p, m, i) -> (m, p, kp, w, s, i)
    return np.ascontiguousarray(pan.transpose(4, 3, 1, 0, 2, 5))


def _pack_w2_f8(w2q):
    """e4m3 (I, D) -> (P, JP8, 2, D) with I zero-padded to 2*JP8*P rows."""
    ii, dd = w2q.shape
    pad = np.zeros((2 * JP8 * P, dd), F8)
    pad[:ii] = w2q
    return np.ascontiguousarray(
        pad.reshape(JP8, 2, P, dd).transpose(2, 0, 1, 3))


def prep(x, weights, indices, W1, W3, W2, Ws1, Ws3, Ws2, cfg):
    """Host-side dispatch: tier split, shard/gather/pad/cast/pre-tile."""
    T, D, E = cfg["T"], cfg["D"], cfg["E"]
    dv = _derived(cfg)
    epc, kd, it_r, it_s = dv["epc"], dv["kd"], dv["it_r"], dv["it_s"]
    nt, sh_tok = dv["n_tok_shards"], dv["sh_tok"]
    sh_half = cfg["sh_half"]

    xf = np.asarray(x, F32).reshape(T, D)
    wts = np.asarray(weights, F32)
    idx = np.asarray(indices).astype(np.int64)
    W1 = np.asarray(W1, F32)
    W3 = np.asarray(W3, F32)
    W2 = np.asarray(W2, F32)
    Ws1 = np.asarray(Ws1, F32)
    Ws3 = np.asarray(Ws3, F32)
    Ws2 = np.asarray(Ws2, F32)

    # Per-(token, expert) combine weight; duplicate expert ids accumulate.
    comb = np.zeros((T, E), F32)
    np.add.at(comb, (np.arange(T)[:, None], idx), wts)

    # Tier split per expert: bf16 tokens (comb >= T_FP8) and fp8 tokens.
    tok_bf = [np.nonzero(comb[:, e] >= T_FP8)[0] for e in range(E)]
    tok_f8 = [np.nonzero((comb[:, e] > 0) & (comb[:, e] < T_FP8))[0]
              for e in range(E)]
    nb = np.array([len(t) for t in tok_bf])
    nf = np.array([len(t) for t in tok_f8])

    # Exact 2-slot assignment: minimize CB0+CB1 + w*(CF0+CF1) over all
    # 16-choose-8 splits (w ~ fp8 column cost relative to bf16).
    import itertools
    best = None
    for s0 in itertools.combinations(range(E), epc * cfg["ncores"] // 2):
        s0 = list(s0)
        s1 = [i for i in range(E) if i not in s0]
        cost = (nb[s0].max() + nb[s1].max()
                + 0.62 * (nf[s0].max() + nf[s1].max()))
        if best is None or cost < best[0]:
            best = (cost, s0, s1)
    eslot = np.array([best[1], best[2]])  # eslot[j][c] = expert id

    CBs = [max(64, -(-int(nb[eslot[j]].max()) // 4) * 4) for j in range(epc)]
    CFs = [max(64, -(-int(nf[eslot[j]].max()) // 16) * 16) for j in range(epc)]

    xT = np.ascontiguousarray(xf.T)                 # (D, T) fp32
    xT16 = xT.astype(BF16)
    xTq = _q8(xT, SX)

    def _xt_layout(cols):
        # (D, n) -> (kd//XG, P, XG, n): one contiguous DMA per k-tile group.
        n = cols.shape[1]
        return np.ascontiguousarray(
            cols.reshape(kd // XG, XG, P, n).swapaxes(1, 2))

    in_maps = []
    for c in range(cfg["ncores"]):
        m = {}
        for j in range(epc):
            e = int(eslot[j][c])
            # bf16 tier
            toks = tok_bf[e]
            tpad = np.zeros(CBs[j], np.int64)
            tpad[:len(toks)] = toks
            m[f"xt{j}"] = _xt_layout(xT16[:, tpad])
            m[f"w13_{j}"] = _panelize_w13(W1[e], W3[e], it_r).astype(BF16)
            m[f"w2_{j}"] = np.ascontiguousarray(
                W2[e].reshape(it_r, P, cfg["d_out"])).astype(BF16)
            cg = np.zeros(CBs[j], F32)
            cg[:len(toks)] = comb[toks, e]
            m[f"cb{j}"] = np.ascontiguousarray(np.broadcast_to(cg, (P, CBs[j])))
            # fp8 tier
            toksf = tok_f8[e]
            tpadf = np.zeros(CFs[j], np.int64)
            tpadf[:len(toksf)] = toksf
            m[f"xf{j}"] = _xt_layout(xTq[:, tpadf])
            m[f"wf13_{j}"] = _panelize_w13_f8(_q8(W1[e], SW), _q8(W3[e], SW), it_r)
            m[f"wf2_{j}"] = _pack_w2_f8(_q8(W2[e], SW2))
            cgf = np.zeros(CFs[j], F32)
            cgf[:len(toksf)] = comb[toksf, e] / (SH * SW2)
            m[f"cf{j}"] = np.ascontiguousarray(np.broadcast_to(cgf, (P, CFs[j])))
        # Shared expert: 2-way inter split x (ncores/2)-way token split.
        h, q = divmod(c, nt)
        m["xts"] = _xt_layout(xT16[:, q * sh_tok:(q + 1) * sh_tok])
        m["ws13"] = _panelize_w13(Ws1[:, h * sh_half:(h + 1) * sh_half],
                                  Ws3[:, h * sh_half:(h + 1) * sh_half],
                                  it_s).astype(BF16)
        m["ws2"] = np.ascontiguousarray(
            Ws2[h * sh_half:(h + 1) * sh_half].reshape(it_s, P, cfg["d_out"])).astype(BF16)
        in_maps.append(m)

    meta = dict(tok_bf=tok_bf, tok_f8=tok_f8, nb=nb, nf=nf,
                CBs=CBs, CFs=CFs, eslot=eslot)
    return in_maps, meta


def combine(results, meta, cfg):
    """Host-side unshard: sum shared partials, scatter-add routed outputs."""
    T, D = cfg["T"], cfg["d_out"]
    dv = _derived(cfg)
    epc, nt, sh_tok = dv["epc"], dv["n_tok_shards"], dv["sh_tok"]
    out = np.zeros((T, D), F32)
    for c in range(cfg["ncores"]):
        r = results[c]
        q = c % nt
        out[q * sh_tok:(q + 1) * sh_tok] += \
            np.asarray(r["z"], F32).reshape(D, sh_tok).T
        for j in range(epc):
            e = int(meta["eslot"][j][c])
            yt = np.asarray(r[f"y{j}"], F32).reshape(D, -1)
            out[meta["tok_bf"][e]] += yt.T[:meta["nb"][e]]
            yf = np.asarray(r[f"yf{j}"], F32).reshape(D, -1)
            out[meta["tok_f8"][e]] += yf.T[:meta["nf"][e]]
    return out


# Test-harness knobs (kernel() callers get no-trace defaults).
TRACE = False
TMPDIR = None
LAST_RESULT = None


def kernel(x, weights, indices, W1, W3, W2, Ws1, Ws3, Ws2):
    global LAST_RESULT
    cfg = FULL_CFG
    in_maps, meta = prep(x, weights, indices, W1, W3, W2,
                         Ws1, Ws3, Ws2, cfg)
    nc = build_program(meta["CBs"], meta["CFs"], cfg)
    res = run_bass_kernel_spmd(nc, in_maps, core_ids=list(range(cfg["ncores"])),
                               trace=TRACE, tmpdir=TMPDIR)
    LAST_RESULT = res
    out = combine(res.results, meta, cfg)
    return out.reshape(1, cfg["T"], cfg["D"]).astype(F32)


# revision 66
# speedup vs baseline: 1.0466x; 1.0048x over previous
"""MoE (DeepSeek-style, no gate) SwiGLU kernel for 8 Trainium2 NeuronCores.

Strategy (expert parallelism, per the sharding hint):
  - 16 routed experts sharded 2-per-core across 8 cores.
  - Token dispatch done host-side: for each expert, gather the tokens routed
    to it (topk membership), pad to a common capacity, and ship the
    pre-transposed activation columns xT[:, tokens] to the owning core.
  - Shared expert tensor-parallel over its inter dim (2816/2 halves) x
    token quarters, computed on all 2048 tokens in bf16.
  - Each core runs the same Bass program (SPMD) on its own shard; the host
    scatter-adds routed outputs and sums shared-expert partials.

Two precision tiers for routed tokens (tier by combine weight):
  - comb >= T_FP8: bf16 pipeline (PE at 1 col/cycle).  The output error
    contribution of a routed pair scales with comb, so large-comb pairs
    need bf16.
  - comb <  T_FP8: fp8 e4m3 pipeline with perf_mode=DoubleRow (2 k-tiles
    per matmul, ~1.4x PE throughput).  All quantization scales are powers
    of two folded into ACT scale operands and the host-side combine
    weights, so the fp8 path has the same instruction structure as bf16.
  Measured (numpy sim of e4m3-quantized pipeline on the real inputs):
  T_FP8=0.45 -> rel err ~1.4e-2 vs the 2e-2 gate; all-bf16 is 3.9e-3.

Compute layout per phase (one phase = one SwiGLU MLP on a token set):
  phase 1:  h1T = W1p.T @ xT, h3T = W3p.T @ xT   (I on partitions, tokens free)
            h' = silu(h1T) * h3T                  -> bf16 (or e4m3) in SBUF
  phase 2:  y[tok_tile] = h'.T @ W2p, scaled per-token by comb weight on
            PSUM eviction (DVE mul with a broadcast comb row).

Overlap structure (each item measured against the 437us all-bf16 baseline;
this version: ~430us, rel err 1.46e-2):
  - Phase interleave e0,f0,e1,f1,shared: big bf16 weight prefetches overlap
    an fp8 phase's compute in addition to a phase 2, and buffer-tag
    rotations (w13/wf13) get a whole phase of slack before reuse.
  - Per-phase alternating A/B tags (bufs=1) for xt/xf/cb: every phase's
    input DMAs are first-use, so the serial SP HWDGE ring streams them in
    demand order instead of head-of-line blocking on buffer-recycle waits.
  - Software-pipelined emission (generators): each phase's DMA header is
    emitted right after the previous phase's phase-1 panels, ahead of its
    w2 bulk on the ring.
  - Output DMAs ride the ACT ring: on the SP ring they pace the next
    phase's inputs to phase-2 compute.
  - Ramp: 11 dummy warm-up matmuls at t=0 keep the PE HAM activity monitor
    busy during the initial DMA wait; the first phase's m=0 W13 panel is
    split into 4 k-group DMAs interleaved with the xt token groups in
    SP-ring order, first two xt groups on the then-idle ACT ring.
Measured dead ends: fp8 for the shared expert or large-comb pairs fails the
2e-2 gate; dropping small-comb pairs outright fails (max-abs metric); int8
matmul unsupported by bass; >2 ACT-ring ramp DMAs stall the PE; w2/wf2 on
the gpsimd SWDGE ring (pushes at t=0 steal ramp HBM bw) or the ACT ring
(gated behind ph1 ACTIVATEs, arrive too late); psA=3/psY=2 PSUM split;
hoisting headers 2 phases early (NaN race); routing later phases' m=0/m=1
through the per-kg ramp tags (scheduler gates w2 pushes on late matmul
counters).
"""

import numpy as np
import ml_dtypes

import concourse.bass as bass
import concourse.bacc as bacc
import concourse.mybir as mybir
import concourse.tile as tile
from concourse.bass_utils import run_bass_kernel_spmd

BF16 = ml_dtypes.bfloat16
F8 = ml_dtypes.float8_e4m3
F32 = np.float32
P = 128
NSZ = 512   # PSUM bank free size (fp32)
XG = 2      # xt k-tiles per DMA group (= one DoubleRow pair)
WKG = 4     # w13 k-tiles per k-group block (bf16 path)

# fp8 tier: threshold and power-of-two quantization scales.
# 0.45 measured 1.4605e-2 vs the 2e-2 gate (sim predicted 1.4507e-2);
# 0.5 saves ~6us of PE streaming (sim: 1.58e-2, margin 1.26x).
T_FP8 = 0.5
SX = 32.0       # x -> e4m3 scale
SW = 1024.0     # W1/W3 -> e4m3 scale
SH = 8.0        # h' -> e4m3 scale
SW2 = 1024.0    # W2 -> e4m3 scale
S1 = SX * SW                  # PSUM p1/p3 carry h*S1
K_COPY = SH / (S1 * S1)       # ACT copy scale: c3 = p3 * K_COPY
KP8 = 8                       # phase-1 k-tile pairs (D/128/2)
JP8 = 6                       # phase-2 I-tile pairs (11 -> pad to 12)

FULL_CFG = dict(
    ncores=8,
    T=2048,
    D=2048,
    E=16,
    I_E=1408,
    sh_half=1408,    # shared expert sharded 2 (inter) x ncores/2 (tokens)
    d_out=2048,
)


def _derived(cfg):
    nt = max(1, cfg["ncores"] // 2)
    return dict(
        epc=cfg["E"] // cfg["ncores"],
        kd=cfg["D"] // P,
        it_r=cfg["I_E"] // P,
        it_s=cfg["sh_half"] // P,
        n_tok_shards=nt,
        sh_tok=cfg["T"] // nt,
    )


def _emit_phase(nc, pools, xt_dram, w13_dram, w2_dram, cb_dram,
                out_rows, n_itiles, cp, cfg, ph, grp, first=False):
    """One bf16 SwiGLU MLP phase over `cp` token columns, `n_itiles` I-tiles.

    Output is TRANSPOSED: out_rows[mt2] is the DRAM destination for model-dim
    tile mt2 ([128, cp] = [D-tile, tokens]).  cb_dram is None for the shared
    expert; otherwise the combine weight broadcast to [128, cp].  `first`
    marks the program's first phase: its m=0 W13 panel DMA is split per
    k-group and interleaved with the xt groups for the ramp.
    """
    kd = _derived(cfg)["kd"]
    d_out = cfg["d_out"]
    dt = mybir.dt.bfloat16
    f32 = mybir.dt.float32
    nkg = kd // WKG

    xtp, wp, hpp, w2p, sp, op, cgp, psA, psY = (
        pools["xt"], pools["w"], pools["hp"], pools["w2"], pools["s"],
        pools["o"], pools["cg"], pools["psA"], pools["psY"])

    # DMA header. The SP HWDGE ring executes DMAs serially in issue order,
    # so for the program's first phase the m=0 panel k-groups and the xt
    # token-k groups are interleaved to match the matmul consumption order.
    xtg = []
    if first:
        wkg = [wp.tile([P, 2, WKG, P], dt, tag=f"w13a{kg}", bufs=2,
                       name=f"w13a_{ph}_{kg}") for kg in range(nkg)]
        wpre = [wkg]
        for g in range(kd // XG):
            xtg.append(xtp.tile([P, XG, cp], dt, tag=f"xt{grp}_{g}",
                                name=f"xt_{ph}_{g}"))
        # First two xt groups ride the otherwise-idle ACT HWDGE ring.
        nc.scalar.dma_start(out=xtg[0][:], in_=xt_dram[0])
        nc.scalar.dma_start(out=xtg[1][:], in_=xt_dram[1])
        # SP ring in PE demand order, weights one k-group ahead.
        for kind, i in (("w", 0), ("w", 1), ("x", 2), ("x", 3), ("w", 2),
                        ("x", 4), ("x", 5), ("w", 3), ("x", 6), ("x", 7)):
            if kind == "w":
                nc.sync.dma_start(out=wkg[i][:], in_=w13_dram[0][:, i])
            else:
                nc.sync.dma_start(out=xtg[i][:], in_=xt_dram[i])
    else:
        wpre = [wp.tile([P, nkg, 2, WKG, P], dt, tag="w13", name=f"w13_{ph}_0")]
        nc.sync.dma_start(out=wpre[0][:], in_=w13_dram[0])
        for g in range(kd // XG):
            xge = xtp.tile([P, XG, cp], dt, tag=f"xt{grp}_{g}",
                           name=f"xt_{ph}_{g}")
            nc.sync.dma_start(out=xge[:], in_=xt_dram[g])
            xtg.append(xge)
    if n_itiles > 1:
        # (Measured dead end: routing phases' m=0/m=1 through per-kg ramp
        # tags made the scheduler gate this phase's w2 pushes on late-phase
        # matmul counters -> 12us stall mid phase-2.)
        w13b = wp.tile([P, nkg, 2, WKG, P], dt, tag="w13", name=f"w13_{ph}_1")
        nc.sync.dma_start(out=w13b[:], in_=w13_dram[1])
        wpre.append(w13b)

    cbt = None
    if cb_dram is not None:
        cbr = cgp.tile([P, cp], f32, tag=f"cbr{grp}", name=f"cbr_{ph}")
        nc.sync.dma_start(out=cbr[:], in_=cb_dram[:])
        # Bounce through DVE so the per-tile eviction muls below need only
        # the PE wait (DVE has already observed the cb DMA here).
        cbt = cgp.tile([P, cp], f32, tag=f"cb{grp}", name=f"cb_{ph}")
        nc.vector.tensor_copy(cbt[:], cbr[:])

    yield  # header done (emitted one phase early so the ring prefetches it)

    # ---- phase 1: h' = silu(xW1) * (xW3), transposed layout [I, tokens] ----
    hp = []
    for m in range(n_itiles):
        if m < len(wpre):
            w13t = wpre[m]
        else:
            w13t = wp.tile([P, nkg, 2, WKG, P], dt, tag="w13", name=f"w13_{ph}_{m}")
            nc.sync.dma_start(out=w13t[:], in_=w13_dram[m])
        hpm = hpp.tile([P, cp], dt, tag=f"hp_{m}", name=f"hp_{ph}_{m}")
        for n0 in range(0, cp, NSZ):
            nsz = min(NSZ, cp - n0)
            p1 = psA.tile([P, nsz], f32, tag="p1", name=f"p1_{ph}_{m}_{n0}")
            p3 = psA.tile([P, nsz], f32, tag="p3", name=f"p3_{ph}_{m}_{n0}")
            for kt in range(kd):
                xs = xtg[kt // XG][:, kt % XG, n0:n0 + nsz]
                if isinstance(w13t, list):   # first phase m=0/1: per-kg tiles
                    w1s = w13t[kt // WKG][:, 0, kt % WKG, :]
                    w3s = w13t[kt // WKG][:, 1, kt % WKG, :]
                else:
                    w1s = w13t[:, kt // WKG, 0, kt % WKG, :]
                    w3s = w13t[:, kt // WKG, 1, kt % WKG, :]
                nc.tensor.matmul(p1[:], w1s, xs,
                                 start=(kt == 0), stop=(kt == kd - 1))
                nc.tensor.matmul(p3[:], w3s, xs,
                                 start=(kt == 0), stop=(kt == kd - 1))
            # silu(h1)*h3 = sigmoid(h1)*h3*h1.
            s = sp.tile([P, nsz], f32, tag="s", name=f"s_{ph}_{m}_{n0}")
            nc.scalar.activation(s[:], p1[:],
                                 mybir.ActivationFunctionType.Sigmoid)
            c3 = sp.tile([P, nsz], f32, tag="c3", name=f"c3_{ph}_{m}_{n0}")
            nc.scalar.copy(c3[:], p3[:])
            t = sp.tile([P, nsz], f32, tag="t", name=f"t_{ph}_{m}_{n0}")
            nc.vector.tensor_mul(t[:], s[:], c3[:])
            nc.vector.tensor_mul(hpm[:, n0:n0 + nsz], t[:], p1[:])
        hp.append(hpm)

    yield  # phase 1 done

    # ---- phase 2: out[tok] = comb * (h'.T @ W2) ----
    # Phase-2 weights on the SP ring.  (Measured dead ends: gpsimd/SWDGE
    # pushes jump the queue at t=0 and steal HBM bandwidth from the ramp;
    # ACT-ring pushes are gated behind phase-1 ACTIVATEs and arrive too
    # late to prefetch.)
    w2t = []
    for kt in range(n_itiles):
        w = w2p.tile([P, d_out], dt, tag=f"w2_{kt}", name=f"w2_{ph}_{kt}")
        nc.sync.dma_start(out=w[:], in_=w2_dram[kt])
        w2t.append(w)

    ps2 = [(psY, "py"), (psY, "py"), (psY, "py"), (psY, "py"),
           (psA, "p1"), (psA, "p1"), (psA, "p3"), (psA, "p3")]
    idx = 0
    for mt2 in range(kd):
        osb = op.tile([P, cp], dt, tag="osb", name=f"osb_{ph}_{mt2}")
        for n0 in range(0, cp, NSZ):
            nn = min(NSZ, cp - n0)
            pool, ptag = ps2[idx % len(ps2)]
            idx += 1
            py = pool.tile([P, nn], f32, tag=ptag, name=f"py_{ph}_{mt2}_{n0}")
            for kt in range(n_itiles):
                nc.tensor.matmul(py[:], w2t[kt][:, mt2 * P:(mt2 + 1) * P],
                                 hp[kt][:, n0:n0 + nn],
                                 start=(kt == 0), stop=(kt == n_itiles - 1))
            if cbt is not None:
                nc.vector.tensor_mul(osb[:, n0:n0 + nn], py[:],
                                     cbt[:, n0:n0 + nn])
            elif idx % 2:
                nc.vector.tensor_copy(osb[:, n0:n0 + nn], py[:])
            else:
                nc.scalar.copy(osb[:, n0:n0 + nn], py[:])
        # Output DMAs ride the ACT ring: they are gated on eviction anyway,
        # and on the SP ring they pace the next phase's inputs to phase-2
        # compute (head-of-line blocking).
        nc.scalar.dma_start(out=out_rows[mt2], in_=osb[:])


def _emit_phase_f8(nc, pools, xf_dram, wf13_dram, wf2_dram, cf_dram,
                   out_rows, n_itiles, cp, cfg, ph, grp):
    """fp8 e4m3 DoubleRow SwiGLU phase over `cp` token columns.

    Same structure as the bf16 phase but every matmul is a DoubleRow pair
    (2 k-tiles per instruction).  PSUM p1/p3 carry h*S1; the quantization
    scales are folded into the ACT scale operands (phase 1) and into the
    host-prepared combine row cf = comb/(SH*SW2) (phase 2).  Phase 2 pads
    the 11 I-tiles to 12 with a zero 12th h' tile (and zero W2 rows).
    """
    kd = _derived(cfg)["kd"]
    d_out = cfg["d_out"]
    f8 = mybir.dt.float8e4
    dt = mybir.dt.bfloat16
    f32 = mybir.dt.float32

    xfp, wfp, hqp, w2fp, sp, op, cgp, psA, psY = (
        pools["xf"], pools["wf"], pools["hq"], pools["w2f"], pools["s"],
        pools["o"], pools["cg"], pools["psA"], pools["psY"])

    # DMA header: xt pair-groups + first two weight panels.
    xfg = []
    for g in range(kd // XG):
        xge = xfp.tile([P, XG, cp], f8, tag=f"xf{grp}_{g}", name=f"xf_{ph}_{g}")
        nc.sync.dma_start(out=xge[:], in_=xf_dram[g])
        xfg.append(xge)
    wpre = []
    for m in range(min(3, n_itiles)):
        wt = wfp.tile([P, KP8, 2, 2, P], f8, tag="wf13", bufs=3,
                      name=f"wf13_{ph}_{m}")
        nc.sync.dma_start(out=wt[:], in_=wf13_dram[m])
        wpre.append(wt)

    cfr = cgp.tile([P, cp], f32, tag=f"cfr{grp}", name=f"cfr_{ph}")
    nc.sync.dma_start(out=cfr[:], in_=cf_dram[:])
    cft = cgp.tile([P, cp], f32, tag=f"cf{grp}", name=f"cf_{ph}")
    nc.vector.tensor_copy(cft[:], cfr[:])

    yield  # header done

    # ---- phase 1: hq = e4m3(silu(h1) * h3 * SH), layout [I-tile, tokens] ----
    hq = hqp.tile([P, JP8 * 2, cp], f8, tag="hq", name=f"hq_{ph}")
    nc.gpsimd.memset(hq[:, n_itiles:, :], 0)   # zero-pad I-tiles 11..
    for m in range(n_itiles):
        if m < len(wpre):
            w13t = wpre[m]
        else:
            w13t = wfp.tile([P, KP8, 2, 2, P], f8, tag="wf13", bufs=3,
                            name=f"wf13_{ph}_{m}")
            nc.sync.dma_start(out=w13t[:], in_=wf13_dram[m])
        for n0 in range(0, cp, NSZ):
            nsz = min(NSZ, cp - n0)
            p1 = psA.tile([P, nsz], f32, tag="p1", name=f"q1_{ph}_{m}_{n0}")
            p3 = psA.tile([P, nsz], f32, tag="p3", name=f"q3_{ph}_{m}_{n0}")
            for kp in range(KP8):
                xs = xfg[kp][:, :, n0:n0 + nsz]
                nc.tensor.matmul(p1[:], w13t[:, kp, 0], xs,
                                 start=(kp == 0), stop=(kp == KP8 - 1),
                                 perf_mode=mybir.MatmulPerfMode.DoubleRow)
                nc.tensor.matmul(p3[:], w13t[:, kp, 1], xs,
                                 start=(kp == 0), stop=(kp == KP8 - 1),
                                 perf_mode=mybir.MatmulPerfMode.DoubleRow)
            # hq = sigmoid(p1/S1) * (p3*K_COPY) * p1 = silu(h1)*h3*SH.
            s = sp.tile([P, nsz], f32, tag="s", name=f"fs_{ph}_{m}_{n0}")
            nc.scalar.activation(s[:], p1[:],
                                 mybir.ActivationFunctionType.Sigmoid,
                                 scale=1.0 / S1)
            c3 = sp.tile([P, nsz], f32, tag="c3", name=f"fc3_{ph}_{m}_{n0}")
            nc.scalar.activation(c3[:], p3[:],
                                 mybir.ActivationFunctionType.Copy,
                                 scale=K_COPY)
            # (Measured dead ends: odd-m accumulators on the psY banks, and
            # t=s*p1-first mul order — both regressed ~15us via scheduler
            # cross-tag serialization, like the w13a-tag reuse attempt.)
            t = sp.tile([P, nsz], f32, tag="t", name=f"ft_{ph}_{m}_{n0}")
            nc.vector.tensor_mul(t[:], s[:], c3[:])
            nc.vector.tensor_mul(hq[:, m, n0:n0 + nsz], t[:], p1[:])

    yield  # phase 1 done

    # Phase-2 weights: one 3MB SP-ring DMA issued after the phase-1 panels
    # (needed only at phase 2; issuing it in the header stalls the panels).
    wf2t = w2fp.tile([P, JP8, 2, d_out], f8, tag="wf2", name=f"wf2_{ph}")
    nc.sync.dma_start(out=wf2t[:], in_=wf2_dram[:])

    # ---- phase 2: out[tok] = (comb/(SH*SW2)) * (hq.T @ W2q) ----
    ps2 = [(psY, "py"), (psY, "py"), (psY, "py"), (psY, "py"),
           (psA, "p1"), (psA, "p1"), (psA, "p3"), (psA, "p3")]
    idx = 0
    for mt2 in range(kd):
        osb = op.tile([P, cp], dt, tag="osb", name=f"fosb_{ph}_{mt2}")
        for n0 in range(0, cp, NSZ):
            nn = min(NSZ, cp - n0)
            pool, ptag = ps2[idx % len(ps2)]
            idx += 1
            py = pool.tile([P, nn], f32, tag=ptag, name=f"fpy_{ph}_{mt2}_{n0}")
            for jp in range(JP8):
                nc.tensor.matmul(py[:], wf2t[:, jp, :, mt2 * P:(mt2 + 1) * P],
                                 hq[:, 2 * jp:2 * jp + 2, n0:n0 + nn],
                                 start=(jp == 0), stop=(jp == JP8 - 1),
                                 perf_mode=mybir.MatmulPerfMode.DoubleRow)
            nc.vector.tensor_mul(osb[:, n0:n0 + nn], py[:],
                                 cft[:, n0:n0 + nn])
        nc.scalar.dma_start(out=out_rows[mt2], in_=osb[:])


def build_program(CBs, CFs, cfg):
    """Per-core Bass program.  CBs[j]/CFs[j] = bf16/fp8 token capacities of
    routed expert slot j."""
    nc = bacc.Bacc()
    dt = mybir.dt.bfloat16
    f8 = mybir.dt.float8e4
    f32 = mybir.dt.float32
    dv = _derived(cfg)
    epc, kd, it_r, it_s = dv["epc"], dv["kd"], dv["it_r"], dv["it_s"]
    sh_tok = dv["sh_tok"]
    d_out = cfg["d_out"]
    nkg = kd // WKG

    ins = {}
    for j in range(epc):
        ins[f"xt{j}"] = nc.dram_tensor(f"xt{j}", [kd // XG, P, XG, CBs[j]], dt, kind="ExternalInput")
        ins[f"w13_{j}"] = nc.dram_tensor(f"w13_{j}", [it_r, P, nkg, 2, WKG, P], dt, kind="ExternalInput")
        ins[f"w2_{j}"] = nc.dram_tensor(f"w2_{j}", [it_r, P, d_out], dt, kind="ExternalInput")
        ins[f"cb{j}"] = nc.dram_tensor(f"cb{j}", [P, CBs[j]], f32, kind="ExternalInput")
        ins[f"xf{j}"] = nc.dram_tensor(f"xf{j}", [kd // XG, P, XG, CFs[j]], f8, kind="ExternalInput")
        ins[f"wf13_{j}"] = nc.dram_tensor(f"wf13_{j}", [it_r, P, KP8, 2, 2, P], f8, kind="ExternalInput")
        ins[f"wf2_{j}"] = nc.dram_tensor(f"wf2_{j}", [P, JP8, 2, d_out], f8, kind="ExternalInput")
        ins[f"cf{j}"] = nc.dram_tensor(f"cf{j}", [P, CFs[j]], f32, kind="ExternalInput")
    ins["xts"] = nc.dram_tensor("xts", [kd // XG, P, XG, sh_tok], dt, kind="ExternalInput")
    ins["ws13"] = nc.dram_tensor("ws13", [it_s, P, nkg, 2, WKG, P], dt, kind="ExternalInput")
    ins["ws2"] = nc.dram_tensor("ws2", [it_s, P, d_out], dt, kind="ExternalInput")

    outs = {}
    for j in range(epc):
        outs[f"y{j}"] = nc.dram_tensor(f"y{j}", [kd, P, CBs[j]], dt, kind="ExternalOutput")
        outs[f"yf{j}"] = nc.dram_tensor(f"yf{j}", [kd, P, CFs[j]], dt, kind="ExternalOutput")
    outs["z"] = nc.dram_tensor("z", [kd, P, sh_tok], dt, kind="ExternalOutput")

    with tile.TileContext(nc) as tc:
        with (
            tc.tile_pool(name="xt", bufs=1) as xtp,
            tc.tile_pool(name="xf", bufs=1) as xfp,
            tc.tile_pool(name="w", bufs=3) as wp,
            tc.tile_pool(name="wf", bufs=2) as wfp,
            tc.tile_pool(name="hp", bufs=1) as hpp,
            tc.tile_pool(name="hq", bufs=2) as hqp,
            tc.tile_pool(name="w2", bufs=1) as w2p,
            tc.tile_pool(name="w2f", bufs=1) as w2fp,
            tc.tile_pool(name="s", bufs=2) as sp,
            tc.tile_pool(name="o", bufs=3) as op,
            tc.tile_pool(name="cg", bufs=1) as cgp,
            tc.tile_pool(name="warm", bufs=1) as wmp,
            tc.tile_pool(name="psA", bufs=2, space="PSUM") as psA,
            tc.tile_pool(name="psY", bufs=4, space="PSUM") as psY,
        ):
            pools = dict(xt=xtp, xf=xfp, w=wp, wf=wfp, hp=hpp, hq=hqp,
                         w2=w2p, w2f=w2fp, s=sp, o=op, cg=cgp,
                         psA=psA, psY=psY)
            # PE warm-up: dummy matmuls with no DMA dependency keep the HAM
            # activity monitor busy during the initial DMA wait.
            wt = wmp.tile([P, NSZ], dt, tag="wt", name="warm_src")
            nc.gpsimd.memset(wt[:], 0)
            wps = psY.tile([P, NSZ], f32, tag="py", name="warm_ps")
            for i in range(11):
                nc.tensor.matmul(wps[:], wt[:, :P], wt[:],
                                 start=True, stop=True)
            # Phase order: both bf16 phases first (their large weight DMAs
            # overlap the large bf16 compute), then the fp8 phases, then the
            # shared expert.
            # Per-phase alternating A/B buffer tags (bufs=1) make every
            # phase's input DMAs first-use (no buffer-recycle waits), so the
            # SP ring streams the whole program's inputs in demand order.
            # Phase interleave bf/f8 + software-pipelined emission: each
            # phase's DMA header is emitted right after the PREVIOUS phase's
            # phase-1 panels, so on the serial SP ring it transfers during
            # that phase's compute instead of queuing behind its w2 bulk.
            gens = []
            for j in range(epc):
                gens.append(_emit_phase(
                    nc, pools, ins[f"xt{j}"], ins[f"w13_{j}"],
                    ins[f"w2_{j}"], ins[f"cb{j}"],
                    [outs[f"y{j}"][mt2] for mt2 in range(kd)],
                    it_r, CBs[j], cfg, ph=f"e{j}", grp="AB"[j % 2],
                    first=(j == 0)))
                gens.append(_emit_phase_f8(
                    nc, pools, ins[f"xf{j}"], ins[f"wf13_{j}"],
                    ins[f"wf2_{j}"], ins[f"cf{j}"],
                    [outs[f"yf{j}"][mt2] for mt2 in range(kd)],
                    it_r, CFs[j], cfg, ph=f"f{j}", grp="AB"[j % 2]))
            gens.append(_emit_phase(
                nc, pools, ins["xts"], ins["ws13"], ins["ws2"], None,
                [outs["z"][mt2] for mt2 in range(kd)],
                it_s, sh_tok, cfg, ph="s", grp="A"))

            def step(g):
                next(g, None)

            # (Measured dead end: hoisting headers 2 phases early produced
            # NaN output — a timing-exposed race — with no gap improvement.)
            step(gens[0])          # hdr p0
            step(gens[0])          # ph1 p0
            for i in range(1, len(gens)):
                step(gens[i])      # hdr p_i
                step(gens[i - 1])  # ph2 p_{i-1}
                step(gens[i])      # ph1 p_i
            step(gens[-1])         # ph2 p_last
    nc.compile()
    return nc


def _panelize_w13(w1, w3, itiles):
    """(D, I) x2 -> (itiles, P, kd//WKG, 2, WKG, P) bf16 panels."""
    dd, ii = w1.shape
    kd = dd // P
    p1 = w1.reshape(kd, P, itiles, P).transpose(2, 1, 0, 3)
    p3 = w3.reshape(kd, P, itiles, P).transpose(2, 1, 0, 3)
    panel = np.stack([p1, p3], axis=2)           # (it, P, 2, kd, P)
    panel = panel.reshape(itiles, P, 2, kd // WKG, WKG, P)
    return np.ascontiguousarray(panel.transpose(0, 1, 3, 2, 4, 5))


def _q8(a, s):
    return np.clip(a * s, -240.0, 240.0).astype(F8)


def _panelize_w13_f8(w1q, w3q, itiles):
    """e4m3 (D, I) x2 -> (itiles, P, KP8, 2{w1,w3}, 2{pair}, P)."""
    pan = np.stack([w1q.reshape(KP8, 2, P, itiles, P),
                    w3q.reshape(KP8, 2, P, itiles, P)], axis=0)
    # (w, kp, s, p, m, i) -> (m, p, kp, w, s, i)
    return np.ascontiguousarray(pan.transpose(4, 3, 1, 0, 2, 5))


def _pack_w2_f8(w2q):
    """e4m3 (I, D) -> (P, JP8, 2, D) with I zero-padded to 2*JP8*P rows."""
    ii, dd = w2q.shape
    pad = np.zeros((2 * JP8 * P, dd), F8)
    pad[:ii] = w2q
    return np.ascontiguousarray(
        pad.reshape(JP8, 2, P, dd).transpose(2, 0, 1, 3))


def prep(x, weights, indices, W1, W3, W2, Ws1, Ws3, Ws2, cfg):
    """Host-side dispatch: tier split, shard/gather/pad/cast/pre-tile."""
    T, D, E = cfg["T"], cfg["D"], cfg["E"]
    dv = _derived(cfg)
    epc, kd, it_r, it_s = dv["epc"], dv["kd"], dv["it_r"], dv["it_s"]
    nt, sh_tok = dv["n_tok_shards"], dv["sh_tok"]
    sh_half = cfg["sh_half"]

    xf = np.asarray(x, F32).reshape(T, D)
    wts = np.asarray(weights, F32)
    idx = np.asarray(indices).astype(np.int64)
    W1 = np.asarray(W1, F32)
    W3 = np.asarray(W3, F32)
    W2 = np.asarray(W2, F32)
    Ws1 = np.asarray(Ws1, F32)
    Ws3 = np.asarray(Ws3, F32)
    Ws2 = np.asarray(Ws2, F32)

    # Per-(token, expert) combine weight; duplicate expert ids accumulate.
    comb = np.zeros((T, E), F32)
    np.add.at(comb, (np.arange(T)[:, None], idx), wts)

    # Tier split per expert: bf16 tokens (comb >= T_FP8) and fp8 tokens.
    tok_bf = [np.nonzero(comb[:, e] >= T_FP8)[0] for e in range(E)]
    tok_f8 = [np.nonzero((comb[:, e] > 0) & (comb[:, e] < T_FP8))[0]
              for e in range(E)]
    nb = np.array([len(t) for t in tok_bf])
    nf = np.array([len(t) for t in tok_f8])

    # Exact 2-slot assignment: minimize CB0+CB1 + w*(CF0+CF1) over all
    # 16-choose-8 splits (w ~ fp8 column cost relative to bf16).
    import itertools
    best = None
    for s0 in itertools.combinations(range(E), epc * cfg["ncores"] // 2):
        s0 = list(s0)
        s1 = [i for i in range(E) if i not in s0]
        cost = (nb[s0].max() + nb[s1].max()
                + 0.62 * (nf[s0].max() + nf[s1].max()))
        if best is None or cost < best[0]:
            best = (cost, s0, s1)
    eslot = np.array([best[1], best[2]])  # eslot[j][c] = expert id

    CBs = [max(64, -(-int(nb[eslot[j]].max()) // 4) * 4) for j in range(epc)]
    CFs = [max(64, -(-int(nf[eslot[j]].max()) // 16) * 16) for j in range(epc)]

    xT = np.ascontiguousarray(xf.T)                 # (D, T) fp32
    xT16 = xT.astype(BF16)
    xTq = _q8(xT, SX)

    def _xt_layout(cols):
        # (D, n) -> (kd//XG, P, XG, n): one contiguous DMA per k-tile group.
        n = cols.shape[1]
        return np.ascontiguousarray(
            cols.reshape(kd // XG, XG, P, n).swapaxes(1, 2))

    in_maps = []
    for c in range(cfg["ncores"]):
        m = {}
        for j in range(epc):
            e = int(eslot[j][c])
            # bf16 tier
            toks = tok_bf[e]
            tpad = np.zeros(CBs[j], np.int64)
            tpad[:len(toks)] = toks
            m[f"xt{j}"] = _xt_layout(xT16[:, tpad])
            m[f"w13_{j}"] = _panelize_w13(W1[e], W3[e], it_r).astype(BF16)
            m[f"w2_{j}"] = np.ascontiguousarray(
                W2[e].reshape(it_r, P, cfg["d_out"])).astype(BF16)
            cg = np.zeros(CBs[j], F32)
            cg[:len(toks)] = comb[toks, e]
            m[f"cb{j}"] = np.ascontiguousarray(np.broadcast_to(cg, (P, CBs[j])))
            # fp8 tier
            toksf = tok_f8[e]
            tpadf = np.zeros(CFs[j], np.int64)
            tpadf[:len(toksf)] = toksf
            m[f"xf{j}"] = _xt_layout(xTq[:, tpadf])
            m[f"wf13_{j}"] = _panelize_w13_f8(_q8(W1[e], SW), _q8(W3[e], SW), it_r)
            m[f"wf2_{j}"] = _pack_w2_f8(_q8(W2[e], SW2))
            cgf = np.zeros(CFs[j], F32)
            cgf[:len(toksf)] = comb[toksf, e] / (SH * SW2)
            m[f"cf{j}"] = np.ascontiguousarray(np.broadcast_to(cgf, (P, CFs[j])))
        # Shared expert: 2-way inter split x (ncores/2)-way token split.
        h, q = divmod(c, nt)
        m["xts"] = _xt_layout(xT16[:, q * sh_tok:(q + 1) * sh_tok])
        m["ws13"] = _panelize_w13(Ws1[:, h * sh_half:(h + 1) * sh_half],
                                  Ws3[:, h * sh_half:(h + 1) * sh_half],
                                  it_s).astype(BF16)
        m["ws2"] = np.ascontiguousarray(
            Ws2[h * sh_half:(h + 1) * sh_half].reshape(it_s, P, cfg["d_out"])).astype(BF16)
        in_maps.append(m)

    meta = dict(tok_bf=tok_bf, tok_f8=tok_f8, nb=nb, nf=nf,
                CBs=CBs, CFs=CFs, eslot=eslot)
    return in_maps, meta


def combine(results, meta, cfg):
    """Host-side unshard: sum shared partials, scatter-add routed outputs."""
    T, D = cfg["T"], cfg["d_out"]
    dv = _derived(cfg)
    epc, nt, sh_tok = dv["epc"], dv["n_tok_shards"], dv["sh_tok"]
    out = np.zeros((T, D), F32)
    for c in range(cfg["ncores"]):
        r = results[c]
        q = c % nt
        out[q * sh_tok:(q + 1) * sh_tok] += \
            np.asarray(r["z"], F32).reshape(D, sh_tok).T
        for j in range(epc):
            e = int(meta["eslot"][j][c])
            yt = np.asarray(r[f"y{j}"], F32).reshape(D, -1)
            out[meta["tok_bf"][e]] += yt.T[:meta["nb"][e]]
            yf = np.asarray(r[f"yf{j}"], F32).reshape(D, -1)
            out[meta["tok_f8"][e]] += yf.T[:meta["nf"][e]]
    return out


# Test-harness knobs (kernel() callers get no-trace defaults).
TRACE = False
TMPDIR = None
LAST_RESULT = None


def kernel(x, weights, indices, W1, W3, W2, Ws1, Ws3, Ws2):
    global LAST_RESULT
    cfg = FULL_CFG
    in_maps, meta = prep(x, weights, indices, W1, W3, W2,
                         Ws1, Ws3, Ws2, cfg)
    nc = build_program(meta["CBs"], meta["CFs"], cfg)
    res = run_bass_kernel_spmd(nc, in_maps, core_ids=list(range(cfg["ncores"])),
                               trace=TRACE, tmpdir=TMPDIR)
    LAST_RESULT = res
    out = combine(res.results, meta, cfg)
    return out.reshape(1, cfg["T"], cfg["D"]).astype(F32)
